# revision 8
# baseline (speedup 1.0000x reference)
"""Bahdanau (additive) attention TRN2 Bass kernel (v3).

reference:
    proj_in = einsum("bse,ea->bsa", inputs, W_in)      # [B,S,A]
    proj_q  = (query @ W_q)[:, None, :]                # [B,1,A]
    scores  = einsum("bsa,a->bs", tanh(proj_in+proj_q), w_att)
    weights = softmax(scores, axis=1)
    context = einsum("bs,bsa->ba", weights, proj_in)   # [B,A]

B,S,E,Q,A = 32,2048,1024,1024,512.

Sharding: data-parallel over batch. 8 cores x 4 batches each; weights
replicated. No collectives; host scatters inputs / gathers outputs.

v3 structure (trace-driven, from the 188.4us baseline and 178.3us v2):
  - All bulk loads share the (fast) sync DMA ring in priority order:
    qT, wq, watt, w_in, then x pairs.  Warmup starts ~4us, main ~9us.
  - x loaded as 4 pair-DMAs per batch ([128, 2, S] tiles): halves the
    DMA instruction count vs per-chunk loads - the end-of-kernel drain
    waits on a serial backlog of per-DMA completion pokes (~16/instr,
    ~20ns each), so fewer instructions directly shrink the tail.
  - Main matmul loop (at, ec-outer, sc-inner): stationary reused, first
    MM needs only x chunk 0.  mm_acc 6 PSUM banks, scores 2.
  - Scores col-tiled into ONE PSUM bank (stripes at partitions
    {0,32,64,96}); bank memset + start=False accumulation avoids the
    whole-bank has_written-clear hazard; at-major order for col-group
    concurrency (deferred) / sc-major to pipeline with the tanh slices
    (final).  exp over the whole stripe bank in one ACT op.
  - Deferred epilogue COMPLETES INSIDE its carrier batch: part1
    (scores/exp/broadcast) after at0, ctx multiplies on GPSIMD after
    at1 (gpsimd is otherwise idle; DVE casts that gate PSUM release are
    never blocked), denominator on DVE after at1, ACT reduces at at2,
    gpsimd normalize + scalar-ring store at at3.  The tail then holds
    ONLY the final batch's epilogue.
  - Final epilogue: PE K=1 ones-matmul broadcast, denominator via tiny
    PE matmuls (mask dot + broadcast, off the DVE chain), ctx TTs split
    DVE/GPSIMD, reduces split ACT/DVE.
"""

import sys

sys.path.insert(0, "/opt/trn_rl_repo")

import ml_dtypes
import numpy as np

import concourse.bass as bass
import concourse.tile as tile
from concourse import bacc, bass_utils, mybir

B, S, E, Q, A = 32, 2048, 1024, 1024, 512
NCORES = 8
BPC = B // NCORES  # batches per core
P = 128
EC = E // P  # 8 e-chunks
QC = Q // P  # 8 q-chunks
AT = A // P  # 4 a-tiles
SF = 512  # matmul moving free dim
SC = S // SF  # 4 s-chunks
QPAD = 128  # padded free dim for the transposed proj_q warmup

BF = mybir.dt.bfloat16
F32 = mybir.dt.float32
TANH = mybir.ActivationFunctionType.Tanh
EXP = mybir.ActivationFunctionType.Exp
COPY = mybir.ActivationFunctionType.Copy


def build():
    nc = bacc.Bacc("TRN2", target_bir_lowering=False, debug=False)

    xT = nc.dram_tensor("xT", [BPC, E, S], BF, kind="ExternalInput")
    qT = nc.dram_tensor("qT", [Q, QPAD], BF, kind="ExternalInput")
    w_in = nc.dram_tensor("w_in", [E, A], BF, kind="ExternalInput")
    w_q = nc.dram_tensor("w_q", [Q, A], BF, kind="ExternalInput")
    w_att = nc.dram_tensor("w_att", [A], BF, kind="ExternalInput")
    out = nc.dram_tensor("out", [BPC, A], F32, kind="ExternalOutput")

    with tile.TileContext(nc) as tc:
        with (
            tc.tile_pool(name="const", bufs=1) as const,
            tc.tile_pool(name="xtp", bufs=2) as xtp,
            tc.tile_pool(name="ttp", bufs=2) as ttp,
            tc.tile_pool(name="small", bufs=3) as small,
            tc.tile_pool(name="mm_ps", bufs=6, space="PSUM") as mm_ps,
            tc.tile_pool(name="sc_ps", bufs=2, space="PSUM") as sc_ps,
            tc.tile_pool(name="dram", bufs=2, space="DRAM") as dram,
        ):
            # ---- loads: one ring (sync), priority order ---------------
            qT_sb = const.tile([P, QC, QPAD], BF)
            nc.sync.dma_start(
                qT_sb,
                bass.AP(tensor=qT, offset=0, ap=[[QPAD, P], [P * QPAD, QC], [1, QPAD]]),
            )
            wq_sb = const.tile([P, QC, A], BF)
            nc.sync.dma_start(
                wq_sb,
                bass.AP(tensor=w_q, offset=0, ap=[[A, P], [P * A, QC], [1, A]]),
            )
            watt_sb = const.tile([P, AT], BF)
            nc.sync.dma_start(watt_sb, w_att.ap().rearrange("(at p) -> p at", p=P))
            w_sb = const.tile([P, EC, AT, P], BF)
            nc.sync.dma_start(
                w_sb,
                bass.AP(tensor=w_in, offset=0, ap=[[A, P], [P * A, EC], [P, AT], [1, P]]),
            )

            ones2 = const.tile([P, P], BF)
            nc.vector.memset(ones2, 1.0)
            ones_f = const.tile([1, P], F32)
            nc.vector.memset(ones_f, 1.0)
            # mask with 1.0 at the score-stripe partitions {0,32,64,96}
            mask_f = const.tile([P, 1], F32)
            nc.vector.memset(mask_f, 0.0)
            for sc in range(SC):
                nc.vector.memset(mask_f[32 * sc : 32 * sc + 1, :], 1.0)

            # ---- proj_q warmup: out[b_pad, a] = sum_q qT[q, b] wq[q, a].
            # One 8-MM N=512 chain (~3.5us) doubles as the HAM warmup.
            pq_ps = mm_ps.tile([P, SF], F32, name="mm_acc")
            for qc in range(QC):
                nc.tensor.matmul(
                    pq_ps,
                    qT_sb[:, qc, :],
                    wq_sb[:, qc, :],
                    start=(qc == 0),
                    stop=(qc == QC - 1),
                )
            pq_flat = small.tile([P, A], F32, name="pq_flat", bufs=1)
            nc.scalar.copy(pq_flat[:BPC, :], pq_ps[:BPC, :])
            pq_dram = dram.tile([A, BPC], F32, name="pq_dram")
            nc.scalar.dma_start(
                bass.AP(
                    tensor=pq_dram.tensor,
                    offset=pq_dram.offset,
                    ap=[[1, BPC], [BPC, A]],
                ),
                pq_flat[:BPC, :],
            )
            projq = const.tile([P, AT, BPC], F32)
            nc.scalar.dma_start(
                projq,
                bass.AP(
                    tensor=pq_dram.tensor,
                    offset=pq_dram.offset,
                    ap=[[BPC, P], [P * BPC, AT], [1, BPC]],
                ),
            )

            # ---- epilogue pieces -------------------------------------
            def emit_scores(pts, sc_major):
                """Col-tiled scores: ONE PSUM bank, 4 stripes at partitions
                {0,32,64,96}; start=False onto a zeroed bank; whole-bank exp
                in one ACT op.  at-major = col-group concurrency (deferred);
                sc-major = pipeline with the carrier's tanh slices (final)."""
                sps = sc_ps.tile([P, SF], F32, name="sps")
                nc.vector.memset(sps, 0.0)
                order = (
                    [(at, sc) for sc in range(SC) for at in range(AT)]
                    if sc_major
                    else [(at, sc) for at in range(AT) for sc in range(SC)]
                )
                for at, sc in order:
                    nc.tensor.matmul(
                        sps[32 * sc : 32 * sc + 1, :],
                        watt_sb[:, at : at + 1],
                        pts[at][:, sc * SF : (sc + 1) * SF],
                        start=False,
                        stop=(at == AT - 1),
                        skip_group_check=True,
                        tile_position=(0, 32 * sc),
                    )
                exp_sb = small.tile([P, SF], BF, name="exp_sb")
                esum = small.tile([P, 1], F32, name="esum")
                nc.scalar.activation(exp_sb, sps, EXP, accum_out=esum)
                return exp_sb, esum

            def emit_wbc_dma(exp_sb):
                # gather stripe rows to DRAM, broadcast back (stride-0 read);
                # dispatched from the scalar ring
                exp_dram = dram.tile([1, S], BF, name="exp_dram")
                nc.scalar.dma_start(
                    bass.AP(
                        tensor=exp_dram.tensor,
                        offset=exp_dram.offset,
                        ap=[[SF, SC], [1, SF]],
                    ),
                    exp_sb[0 : 32 * SC - 31 : 32, :],
                )
                wbc = ttp.tile([P, S], BF, name="wbc")
                nc.scalar.dma_start(
                    wbc,
                    bass.AP(
                        tensor=exp_dram.tensor,
                        offset=exp_dram.offset,
                        ap=[[0, P], [1, S]],
                    ),
                )
                return wbc

            # ---- main batch loop -------------------------------------
            prev = None  # (batch_idx, t tiles, projTall)
            ep = {}  # in-flight deferred epilogue state
            for b in range(BPC):
                xpairs = []
                for h in range(EC // 2):
                    xp = xtp.tile([P, 2, S], BF, name=f"xp{h}")
                    nc.sync.dma_start(
                        xp,
                        bass.AP(
                            tensor=xT,
                            offset=(b * E + h * 2 * P) * S,
                            ap=[[S, P], [P * S, 2], [1, S]],
                        ),
                    )
                    xpairs.append(xp)

                def xchunk(ec):
                    return xpairs[ec // 2][:, ec % 2, :]

                ts_ = []
                projTall = ttp.tile([P, AT * S], BF, name="projTall", bufs=3)
                for at in range(AT):
                    t_sb = ttp.tile([P, S], BF, name=f"t{at}")
                    pss = [mm_ps.tile([P, SF], F32, name="mm_acc") for _ in range(SC)]
                    for ec in range(EC):
                        for sc in range(SC):
                            nc.tensor.matmul(
                                pss[sc],
                                w_sb[:, ec, at, :],
                                xchunk(ec)[:, sc * SF : (sc + 1) * SF],
                                start=(ec == 0),
                                stop=(ec == EC - 1),
                            )
                    for sc in range(SC):
                        sl = slice(at * S + sc * SF, at * S + (sc + 1) * SF)
                        # single PSUM reader (DVE cast) gates PSUM release;
                        # tanh reads the SBUF copy with the proj_q bias fused
                        nc.vector.tensor_copy(projTall[:, sl], pss[sc])
                        nc.scalar.activation(
                            t_sb[:, sc * SF : (sc + 1) * SF],
                            projTall[:, sl],
                            TANH,
                            bias=projq[:, at, b : b + 1],
                        )
                    ts_.append(t_sb)

                    if prev is not None:
                        if at == 0:
                            # part 1: scores / exp / broadcast
                            ep["b"], ep["ts"], ep["proj"] = prev
                            ep["exp"], _ = emit_scores(ep["ts"], sc_major=False)
                            ep["wbc"] = emit_wbc_dma(ep["exp"])
                            ep["cscr"] = [
                                ttp.tile([P, S], BF, name=f"cscr{i}", bufs=1)
                                for i in range(AT)
                            ]
                        if at == 1:
                            # part 2: ctx multiplies on GPSIMD (idle engine -
                            # DVE casts never wait); denominator on DVE
                            for i in range(AT):
                                nc.gpsimd.tensor_tensor(
                                    out=ep["cscr"][i],
                                    in0=ep["proj"][:, i * S : (i + 1) * S],
                                    in1=ep["wbc"],
                                    op=mybir.AluOpType.mult,
                                )
                            tot = small.tile([P, 1], F32, name="tot")
                            nc.vector.tensor_reduce(
                                tot,
                                ep["wbc"],
                                axis=mybir.AxisListType.X,
                                op=mybir.AluOpType.add,
                            )
                            ep["rcp"] = small.tile([P, 1], F32, name="rcp")
                            nc.vector.reciprocal(ep["rcp"], tot)
                        if at == 2:
                            # part 3: ACT reduces
                            ep["c"] = small.tile([P, AT], F32, name="c")
                            for i in range(AT):
                                nc.scalar.activation(
                                    ep["cscr"][i],
                                    ep["cscr"][i],
                                    COPY,
                                    accum_out=ep["c"][:, i : i + 1],
                                )
                        if at == 3:
                            # part 4: normalize on GPSIMD, store on scalar ring
                            for i in range(AT):
                                nc.gpsimd.tensor_scalar_mul(
                                    ep["c"][:, i : i + 1],
                                    ep["c"][:, i : i + 1],
                                    ep["rcp"],
                                )
                            nc.scalar.dma_start(
                                bass.AP(
                                    tensor=out,
                                    offset=ep["b"] * A,
                                    ap=[[1, P], [P, AT]],
                                ),
                                ep["c"],
                            )

                prev = (b, ts_, projTall)

            # ---- final epilogue (the only work left after the last MM) --
            pb, pts, pproj = prev
            exp_sb, esum = emit_scores(pts, sc_major=True)
            # denominator via tiny PE matmuls: tot = mask . esum, then
            # broadcast tot with a K=1 ones matmul; reciprocal on [128,1]
            tot_ps = mm_ps.tile([P, SF], F32, name="mm_acc")
            nc.tensor.matmul(tot_ps[:1, :1], mask_f, esum, start=True, stop=True)
            tot_sb = small.tile([1, 1], F32, name="tot_sb")
            nc.scalar.copy(tot_sb, tot_ps[:1, :1])
            totbc_ps = mm_ps.tile([P, SF], F32, name="mm_acc")
            nc.tensor.matmul(totbc_ps[:, :1], ones_f, tot_sb, start=True, stop=True)
            # PE K=1 ones-matmul broadcast of the exp rows
            wbc = ttp.tile([P, S], BF, name="wbc")
            wpss = []
            for sc in range(SC):
                wps = mm_ps.tile([P, SF], F32, name="mm_acc")
                nc.tensor.matmul(
                    wps,
                    ones2[32 * sc : 32 * sc + 1, :],
                    exp_sb[32 * sc : 32 * sc + 1, :],
                    start=True,
                    stop=True,
                    tile_position=(32 * sc, 0),
                )
                wpss.append(wps)
            for sc in range(SC):
                dst = wbc[:, sc * SF : (sc + 1) * SF]
                if sc % 2 == 0:
                    nc.vector.tensor_copy(dst, wpss[sc])
                else:
                    nc.scalar.copy(dst, wpss[sc])
            totbc = small.tile([P, 1], F32, name="totbc")
            nc.vector.tensor_copy(totbc, totbc_ps[:, :1])
            rcp = small.tile([P, 1], F32, name="rcp")
            nc.vector.reciprocal(rcp, totbc)
            # ctx: at0 chunked on DVE (starts at first wbc chunk), at1 on
            # DVE, at2/at3 on GPSIMD; reduces r0/r2 on ACT, r1/r3 on DVE
            cscrs = [ttp.tile([P, S], BF, name=f"cscr{i}", bufs=1) for i in range(AT)]
            c = small.tile([P, AT], F32, name="c")
            for sc in range(SC):
                sl = slice(sc * SF, (sc + 1) * SF)
                nc.vector.tensor_tensor(
                    out=cscrs[0][:, sl],
                    in0=pproj[:, sc * SF : (sc + 1) * SF],
                    in1=wbc[:, sl],
                    op=mybir.AluOpType.mult,
                )
            nc.vector.tensor_tensor(
                out=cscrs[1], in0=pproj[:, S : 2 * S], in1=wbc,
                op=mybir.AluOpType.mult,
            )
            for i in (2, 3):
                nc.gpsimd.tensor_tensor(
                    out=cscrs[i],
                    in0=pproj[:, i * S : (i + 1) * S],
                    in1=wbc,
                    op=mybir.AluOpType.mult,
                )
            nc.scalar.activation(cscrs[0], cscrs[0], COPY, accum_out=c[:, 0:1])
            nc.vector.tensor_reduce(
                c[:, 1:2], cscrs[1], axis=mybir.AxisListType.X, op=mybir.AluOpType.add
            )
            nc.scalar.activation(cscrs[2], cscrs[2], COPY, accum_out=c[:, 2:3])
            nc.vector.tensor_reduce(
                c[:, 3:4], cscrs[3], axis=mybir.AxisListType.X, op=mybir.AluOpType.add
            )
            for at in range(AT):
                nc.vector.tensor_scalar_mul(c[:, at : at + 1], c[:, at : at + 1], rcp)
            nc.sync.dma_start(
                bass.AP(tensor=out, offset=pb * A, ap=[[1, P], [P, AT]]),
                c,
            )

    nc.compile()
    return nc


_nc = None


def kernel(inputs, query, W_in, W_q, w_att):
    global _nc
    if _nc is None:
        _nc = build()

    bf = ml_dtypes.bfloat16
    x_bf = np.asarray(inputs).astype(bf)
    xT_bf = np.ascontiguousarray(x_bf.transpose(0, 2, 1))
    w_in_bf = np.ascontiguousarray(np.asarray(W_in).astype(bf))
    w_q_bf = np.ascontiguousarray(np.asarray(W_q).astype(bf))
    w_att_bf = np.ascontiguousarray(np.asarray(w_att).astype(bf))

    in_maps = []
    for c in range(NCORES):
        sl = slice(c * BPC, (c + 1) * BPC)
        qTp = np.zeros((Q, QPAD), dtype=bf)
        qTp[:, :BPC] = np.asarray(query[sl]).astype(bf).T
        in_maps.append(
            {
                "xT": np.ascontiguousarray(xT_bf[sl]),
                "qT": qTp,
                "w_in": w_in_bf,
                "w_q": w_q_bf,
                "w_att": w_att_bf,
            }
        )

    res = bass_utils.run_bass_kernel_spmd(_nc, in_maps, core_ids=list(range(NCORES)))
    return np.concatenate([r["out"] for r in res.results], axis=0)


if __name__ == "__main__":
    rng = np.random.default_rng(0)
    ins = {
        "inputs": rng.standard_normal((B, S, E), dtype=np.float32),
        "query": rng.standard_normal((B, Q), dtype=np.float32),
        "W_in": (rng.standard_normal((E, A), dtype=np.float32) / np.sqrt(E)).astype(
            np.float32
        ),
        "W_q": (rng.standard_normal((Q, A), dtype=np.float32) / np.sqrt(Q)).astype(
            np.float32
        ),
        "w_att": (rng.standard_normal((A,), dtype=np.float32) / np.sqrt(A)).astype(
            np.float32
        ),
    }
    got = kernel(**ins)
    print("out shape", got.shape, got.dtype)


# revision 10
# speedup vs baseline: 1.1454x; 1.1454x over previous
"""Bahdanau (additive) attention TRN2 Bass kernel (v4).

reference:
    proj_in = einsum("bse,ea->bsa", inputs, W_in)      # [B,S,A]
    proj_q  = (query @ W_q)[:, None, :]                # [B,1,A]
    scores  = einsum("bsa,a->bs", tanh(proj_in+proj_q), w_att)
    weights = softmax(scores, axis=1)
    context = einsum("bs,bsa->ba", weights, proj_in)   # [B,A]

B,S,E,Q,A = 32,2048,1024,1024,512.

Sharding: data-parallel over batch. 8 cores x 4 batches each; weights
replicated. No collectives; host scatters inputs / gathers outputs.

v4 (trace-driven; baseline 188.4us, v2 178.3us, v3 regression 223us):
  - Only sync+scalar rings are hardware-DGE; gpsimd DMA is software (v2
    put qT/wq there -> 17us startup stall).  v4: sync ring = qT, wq,
    watt, then x pair-loads; scalar ring = w_in first, then epilogue
    bounces/stores.  Warmup ~4us, main ~7us.
  - x as pair-DMAs [128,2,S]: halves DMA instruction count (end-of-
    kernel drain waits on ~16 completion pokes per DMA instruction).
  - gpsimd tensor_tensor measured 4.1us per [128,2048] (3.3x DVE) - v3
    put the ctx multiplies there and regressed; v4 keeps them on DVE,
    gpsimd only does the tiny normalize muls.
  - Main loop (at, ec-outer, sc-inner); mm_acc 6 banks, scores 2.
  - Scores col-tiled into ONE PSUM bank (stripes {0,32,64,96}, memset +
    start=False accumulation); at-major for col-group concurrency
    (~0.4us for 16 MMs, measured); sc-major for the final batch so the
    stripes pipeline with at3's tanh slices.  Whole-bank exp, one ACT op.
  - Deferred epilogues: scores after at0, DVE ctx-mults spread one per
    at-group boundary (never ahead of the PSUM-release casts), ACT
    reduces late for b0/b1 but b2's epilogue runs on an EARLY schedule
    inside b3 so the tail holds only b3's own epilogue.
  - Final epilogue: PE K=1 broadcast; denominator via mask-dot + ones-
    broadcast matmuls on the idle PE; ctx TTs DVE; reduces r0/r2 ACT,
    r1/r3 DVE.
"""

import sys

sys.path.insert(0, "/opt/trn_rl_repo")

import ml_dtypes
import numpy as np

import concourse.bass as bass
import concourse.tile as tile
from concourse import bacc, bass_utils, mybir

B, S, E, Q, A = 32, 2048, 1024, 1024, 512
NCORES = 8
BPC = B // NCORES  # batches per core
P = 128
EC = E // P  # 8 e-chunks
QC = Q // P  # 8 q-chunks
AT = A // P  # 4 a-tiles
SF = 512  # matmul moving free dim
SC = S // SF  # 4 s-chunks
QPAD = 128  # padded free dim for the transposed proj_q warmup

BF = mybir.dt.bfloat16
F32 = mybir.dt.float32
TANH = mybir.ActivationFunctionType.Tanh
EXP = mybir.ActivationFunctionType.Exp
COPY = mybir.ActivationFunctionType.Copy


def build():
    nc = bacc.Bacc("TRN2", target_bir_lowering=False, debug=False)

    xT = nc.dram_tensor("xT", [BPC, E, S], BF, kind="ExternalInput")
    qT = nc.dram_tensor("qT", [Q, QPAD], BF, kind="ExternalInput")
    w_in = nc.dram_tensor("w_in", [E, A], BF, kind="ExternalInput")
    w_q = nc.dram_tensor("w_q", [Q, A], BF, kind="ExternalInput")
    w_att = nc.dram_tensor("w_att", [A], BF, kind="ExternalInput")
    out = nc.dram_tensor("out", [BPC, A], F32, kind="ExternalOutput")

    with tile.TileContext(nc) as tc:
        with (
            tc.tile_pool(name="const", bufs=1) as const,
            tc.tile_pool(name="xtp", bufs=2) as xtp,
            tc.tile_pool(name="ttp", bufs=2) as ttp,
            tc.tile_pool(name="small", bufs=3) as small,
            tc.tile_pool(name="mm_ps", bufs=6, space="PSUM") as mm_ps,
            tc.tile_pool(name="sc_ps", bufs=2, space="PSUM") as sc_ps,
            tc.tile_pool(name="dram", bufs=2, space="DRAM") as dram,
        ):
            # ---- loads: sync ring = qT, wq, watt, x pairs; scalar ring =
            # w_in first, then epilogue traffic
            qT_sb = const.tile([P, QC, QPAD], BF)
            nc.sync.dma_start(
                qT_sb,
                bass.AP(tensor=qT, offset=0, ap=[[QPAD, P], [P * QPAD, QC], [1, QPAD]]),
            )
            wq_sb = const.tile([P, QC, A], BF)
            nc.sync.dma_start(
                wq_sb,
                bass.AP(tensor=w_q, offset=0, ap=[[A, P], [P * A, QC], [1, A]]),
            )
            watt_sb = const.tile([P, AT], BF)
            nc.sync.dma_start(watt_sb, w_att.ap().rearrange("(at p) -> p at", p=P))
            w_sb = const.tile([P, EC, AT, P], BF)
            nc.scalar.dma_start(
                w_sb,
                bass.AP(tensor=w_in, offset=0, ap=[[A, P], [P * A, EC], [P, AT], [1, P]]),
            )

            ones2 = const.tile([P, P], BF)
            nc.vector.memset(ones2, 1.0)
            ones_f = const.tile([1, P], F32)
            nc.vector.memset(ones_f, 1.0)
            # mask with 1.0 at the score-stripe partitions {0,32,64,96}
            mask_f = const.tile([P, 1], F32)
            nc.vector.memset(mask_f, 0.0)
            for sc in range(SC):
                nc.vector.memset(mask_f[32 * sc : 32 * sc + 1, :], 1.0)

            # ---- proj_q warmup: out[b_pad, a] = sum_q qT[q, b] wq[q, a].
            pq_ps = mm_ps.tile([P, SF], F32, name="mm_acc")
            for qc in range(QC):
                nc.tensor.matmul(
                    pq_ps,
                    qT_sb[:, qc, :],
                    wq_sb[:, qc, :],
                    start=(qc == 0),
                    stop=(qc == QC - 1),
                )
            pq_flat = small.tile([P, A], F32, name="pq_flat", bufs=1)
            nc.scalar.copy(pq_flat[:BPC, :], pq_ps[:BPC, :])
            pq_dram = dram.tile([A, BPC], F32, name="pq_dram")
            nc.scalar.dma_start(
                bass.AP(
                    tensor=pq_dram.tensor,
                    offset=pq_dram.offset,
                    ap=[[1, BPC], [BPC, A]],
                ),
                pq_flat[:BPC, :],
            )
            projq = const.tile([P, AT, BPC], F32)
            nc.scalar.dma_start(
                projq,
                bass.AP(
                    tensor=pq_dram.tensor,
                    offset=pq_dram.offset,
                    ap=[[BPC, P], [P * BPC, AT], [1, BPC]],
                ),
            )

            # ---- epilogue pieces -------------------------------------
            def emit_scores(pts, sc_major):
                sps = sc_ps.tile([P, SF], F32, name="sps")
                nc.vector.memset(sps, 0.0)
                order = (
                    [(at, sc) for sc in range(SC) for at in range(AT)]
                    if sc_major
                    else [(at, sc) for at in range(AT) for sc in range(SC)]
                )
                for at, sc in order:
                    nc.tensor.matmul(
                        sps[32 * sc : 32 * sc + 1, :],
                        watt_sb[:, at : at + 1],
                        pts[at][:, sc * SF : (sc + 1) * SF],
                        start=False,
                        stop=(at == AT - 1),
                        skip_group_check=True,
                        tile_position=(0, 32 * sc),
                    )
                exp_sb = small.tile([P, SF], BF, name="exp_sb")
                esum = small.tile([P, 1], F32, name="esum")
                nc.scalar.activation(exp_sb, sps, EXP, accum_out=esum)
                return exp_sb, esum

            def emit_wbc_dma(exp_sb):
                exp_dram = dram.tile([1, S], BF, name="exp_dram")
                nc.scalar.dma_start(
                    bass.AP(
                        tensor=exp_dram.tensor,
                        offset=exp_dram.offset,
                        ap=[[SF, SC], [1, SF]],
                    ),
                    exp_sb[0 : 32 * SC - 31 : 32, :],
                )
                wbc = ttp.tile([P, S], BF, name="wbc")
                nc.scalar.dma_start(
                    wbc,
                    bass.AP(
                        tensor=exp_dram.tensor,
                        offset=exp_dram.offset,
                        ap=[[0, P], [1, S]],
                    ),
                )
                return wbc

            def emit_tt(epi, i):
                nc.vector.tensor_tensor(
                    out=epi["cscr"][i],
                    in0=epi["proj"][:, i * S : (i + 1) * S],
                    in1=epi["wbc"],
                    op=mybir.AluOpType.mult,
                )

            def emit_tot(epi):
                tot = small.tile([P, 1], F32, name="tot")
                nc.vector.tensor_reduce(
                    tot, epi["wbc"], axis=mybir.AxisListType.X, op=mybir.AluOpType.add
                )
                epi["rcp"] = small.tile([P, 1], F32, name="rcp")
                nc.vector.reciprocal(epi["rcp"], tot)

            def emit_red_act(epi, i):
                nc.scalar.activation(
                    epi["cscr"][i], epi["cscr"][i], COPY,
                    accum_out=epi["c"][:, i : i + 1],
                )

            def emit_store(epi):
                for i in range(AT):
                    nc.gpsimd.tensor_scalar_mul(
                        epi["c"][:, i : i + 1], epi["c"][:, i : i + 1], epi["rcp"]
                    )
                nc.scalar.dma_start(
                    bass.AP(tensor=out, offset=epi["b"] * A, ap=[[1, P], [P, AT]]),
                    epi["c"],
                )

            def epi_part1(prv):
                epi = {}
                epi["b"], epi["ts"], epi["proj"] = prv
                epi["exp"], _ = emit_scores(epi["ts"], sc_major=False)
                epi["wbc"] = emit_wbc_dma(epi["exp"])
                epi["cscr"] = [
                    ttp.tile([P, S], BF, name=f"cscr{i}", bufs=1) for i in range(AT)
                ]
                epi["c"] = small.tile([P, AT], F32, name="c")
                return epi

            # ---- main batch loop -------------------------------------
            # epilogue for batch b-1 is emitted inside batch b ("carrier").
            # b0/b1 use the LATE schedule (reduces spill into the next
            # carrier); b2 (carried by the last batch) uses the EARLY
            # schedule so the tail only holds b3's own epilogue.
            prev = None
            ep = None  # epilogue in LATE flight
            ep_old = None  # late-schedule epilogue finishing in this carrier
            for b in range(BPC):
                last = b == BPC - 1
                xpairs = []
                for h in range(EC // 2):
                    xp = xtp.tile([P, 2, S], BF, name=f"xp{h}")
                    nc.sync.dma_start(
                        xp,
                        bass.AP(
                            tensor=xT,
                            offset=(b * E + h * 2 * P) * S,
                            ap=[[S, P], [P * S, 2], [1, S]],
                        ),
                    )
                    xpairs.append(xp)

                ts_ = []
                projTall = ttp.tile([P, AT * S], BF, name="projTall", bufs=3)
                for at in range(AT):
                    t_sb = ttp.tile([P, S], BF, name=f"t{at}")
                    pss = [mm_ps.tile([P, SF], F32, name="mm_acc") for _ in range(SC)]
                    for ec in range(EC):
                        for sc in range(SC):
                            nc.tensor.matmul(
                                pss[sc],
                                w_sb[:, ec, at, :],
                                xpairs[ec // 2][:, ec % 2, sc * SF : (sc + 1) * SF],
                                start=(ec == 0),
                                stop=(ec == EC - 1),
                            )
                    for sc in range(SC):
                        sl = slice(at * S + sc * SF, at * S + (sc + 1) * SF)
                        nc.vector.tensor_copy(projTall[:, sl], pss[sc])
                        nc.scalar.activation(
                            t_sb[:, sc * SF : (sc + 1) * SF],
                            projTall[:, sl],
                            TANH,
                            bias=projq[:, at, b : b + 1],
                        )
                    ts_.append(t_sb)

                    # ---- epilogue emission points (after this group's
                    # drains, so the DVE/ACT queue order keeps PSUM-release
                    # casts and tanhs ahead of epilogue work)
                    if prev is not None:
                        if not last:
                            # LATE schedule (b0/b1): TTs spread one per
                            # group boundary; reduces finish in the NEXT
                            # carrier (ep_old)
                            if at == 0:
                                ep = epi_part1(prev)
                            elif at == 1:
                                emit_tt(ep, 0)
                            elif at == 2:
                                emit_tt(ep, 1)
                                emit_tot(ep)
                            else:
                                emit_tt(ep, 2)
                                emit_tt(ep, 3)
                        else:
                            # EARLY schedule (b2 carried by the last batch):
                            # everything except r2/r3+store lands before the
                            # final epilogue begins
                            if at == 0:
                                ep = epi_part1(prev)
                                emit_tt(ep, 0)
                            elif at == 1:
                                emit_tt(ep, 1)
                                emit_tt(ep, 2)
                            elif at == 2:
                                emit_tt(ep, 3)
                                emit_tot(ep)
                                emit_red_act(ep, 0)
                                emit_red_act(ep, 1)
                    if ep_old is not None:
                        # previous late epilogue's reduces + store, spread
                        # across this carrier's group boundaries
                        if at == 0:
                            emit_red_act(ep_old, 0)
                            emit_red_act(ep_old, 1)
                        elif at == 1:
                            emit_red_act(ep_old, 2)
                            emit_red_act(ep_old, 3)
                        elif at == 2:
                            emit_store(ep_old)
                            ep_old = None

                if not last:
                    ep_old = ep if prev is not None else None
                prev = (b, ts_, projTall)

            # b2 epilogue leftovers (EARLY schedule): reduces r2/r3 + store
            # are emitted after the final part1 so they fill ACT/DVE idle
            # slots without delaying the final exp
            pb, pts, pproj = prev
            exp_sb, esum = emit_scores(pts, sc_major=True)
            # denominator via tiny PE matmuls
            tot_ps = mm_ps.tile([P, SF], F32, name="mm_acc")
            nc.tensor.matmul(tot_ps[:1, :1], mask_f, esum, start=True, stop=True)
            tot_sb = small.tile([1, 1], F32, name="tot_sb")
            nc.scalar.copy(tot_sb, tot_ps[:1, :1])
            totbc_ps = mm_ps.tile([P, SF], F32, name="mm_acc")
            nc.tensor.matmul(totbc_ps[:, :1], ones_f, tot_sb, start=True, stop=True)
            # PE K=1 broadcast of exp rows
            wbc = ttp.tile([P, S], BF, name="wbc")
            wpss = []
            for sc in range(SC):
                wps = mm_ps.tile([P, SF], F32, name="mm_acc")
                nc.tensor.matmul(
                    wps,
                    ones2[32 * sc : 32 * sc + 1, :],
                    exp_sb[32 * sc : 32 * sc + 1, :],
                    start=True,
                    stop=True,
                    tile_position=(32 * sc, 0),
                )
                wpss.append(wps)
            for sc in range(SC):
                dst = wbc[:, sc * SF : (sc + 1) * SF]
                if sc % 2 == 0:
                    nc.vector.tensor_copy(dst, wpss[sc])
                else:
                    nc.scalar.copy(dst, wpss[sc])
            totbc = small.tile([P, 1], F32, name="totbc")
            nc.vector.tensor_copy(totbc, totbc_ps[:, :1])
            rcp = small.tile([P, 1], F32, name="rcp")
            nc.vector.reciprocal(rcp, totbc)

            # b2's remaining reduces fill the gap while final TTs run
            if ep is not None:
                emit_red_act(ep, 2)

            # final ctx: TT0 chunked (starts at first wbc chunk), then rows
            cscrs = [ttp.tile([P, S], BF, name=f"cscr{i}", bufs=1) for i in range(AT)]
            c = small.tile([P, AT], F32, name="c")
            for sc in range(SC):
                sl = slice(sc * SF, (sc + 1) * SF)
                nc.vector.tensor_tensor(
                    out=cscrs[0][:, sl],
                    in0=pproj[:, sc * SF : (sc + 1) * SF],
                    in1=wbc[:, sl],
                    op=mybir.AluOpType.mult,
                )
            for i in (1, 2, 3):
                nc.vector.tensor_tensor(
                    out=cscrs[i],
                    in0=pproj[:, i * S : (i + 1) * S],
                    in1=wbc,
                    op=mybir.AluOpType.mult,
                )
            if ep is not None:
                emit_red_act(ep, 3)
                emit_store(ep)
            nc.scalar.activation(cscrs[0], cscrs[0], COPY, accum_out=c[:, 0:1])
            nc.vector.tensor_reduce(
                c[:, 1:2], cscrs[1], axis=mybir.AxisListType.X, op=mybir.AluOpType.add
            )
            nc.scalar.activation(cscrs[2], cscrs[2], COPY, accum_out=c[:, 2:3])
            nc.vector.tensor_reduce(
                c[:, 3:4], cscrs[3], axis=mybir.AxisListType.X, op=mybir.AluOpType.add
            )
            for at in range(AT):
                nc.vector.tensor_scalar_mul(c[:, at : at + 1], c[:, at : at + 1], rcp)
            nc.sync.dma_start(
                bass.AP(tensor=out, offset=pb * A, ap=[[1, P], [P, AT]]),
                c,
            )

    nc.compile()
    return nc


_nc = None


def kernel(inputs, query, W_in, W_q, w_att):
    global _nc
    if _nc is None:
        _nc = build()

    bf = ml_dtypes.bfloat16
    x_bf = np.asarray(inputs).astype(bf)
    xT_bf = np.ascontiguousarray(x_bf.transpose(0, 2, 1))
    w_in_bf = np.ascontiguousarray(np.asarray(W_in).astype(bf))
    w_q_bf = np.ascontiguousarray(np.asarray(W_q).astype(bf))
    w_att_bf = np.ascontiguousarray(np.asarray(w_att).astype(bf))

    in_maps = []
    for c in range(NCORES):
        sl = slice(c * BPC, (c + 1) * BPC)
        qTp = np.zeros((Q, QPAD), dtype=bf)
        qTp[:, :BPC] = np.asarray(query[sl]).astype(bf).T
        in_maps.append(
            {
                "xT": np.ascontiguousarray(xT_bf[sl]),
                "qT": qTp,
                "w_in": w_in_bf,
                "w_q": w_q_bf,
                "w_att": w_att_bf,
            }
        )

    res = bass_utils.run_bass_kernel_spmd(_nc, in_maps, core_ids=list(range(NCORES)))
    return np.concatenate([r["out"] for r in res.results], axis=0)


if __name__ == "__main__":
    rng = np.random.default_rng(0)
    ins = {
        "inputs": rng.standard_normal((B, S, E), dtype=np.float32),
        "query": rng.standard_normal((B, Q), dtype=np.float32),
        "W_in": (rng.standard_normal((E, A), dtype=np.float32) / np.sqrt(E)).astype(
            np.float32
        ),
        "W_q": (rng.standard_normal((Q, A), dtype=np.float32) / np.sqrt(Q)).astype(
            np.float32
        ),
        "w_att": (rng.standard_normal((A,), dtype=np.float32) / np.sqrt(A)).astype(
            np.float32
        ),
    }
    got = kernel(**ins)
    print("out shape", got.shape, got.dtype)


# revision 11
# speedup vs baseline: 1.1700x; 1.0215x over previous
"""Bahdanau (additive) attention TRN2 Bass kernel (v5).

reference:
    proj_in = einsum("bse,ea->bsa", inputs, W_in)      # [B,S,A]
    proj_q  = (query @ W_q)[:, None, :]                # [B,1,A]
    scores  = einsum("bsa,a->bs", tanh(proj_in+proj_q), w_att)
    weights = softmax(scores, axis=1)
    context = einsum("bs,bsa->ba", weights, proj_in)   # [B,A]

B,S,E,Q,A = 32,2048,1024,1024,512.

Sharding: data-parallel over batch. 8 cores x 4 batches each; weights
replicated. No collectives; host scatters inputs / gathers outputs.

v5 = the measured-best v2 structure (178.3us) + five local deltas:
  1. qT/wq/watt loads moved from the gpsimd ring (software DGE - they
     completed at ~17us and stalled the warmup) to the HEAD of the sync
     hardware ring, before the x loads.  w_in stays on the scalar ring.
  2. x loaded as pair-DMAs [128,2,S]: 16 instead of 32 DMA instructions
     (the end-of-kernel drain serially retires ~16 completion pokes per
     DMA instruction at ~20ns).
  3. Final softmax denominator via tiny PE matmuls (mask-dot of the exp
     accumulator + K=1 ones broadcast) instead of a 2.27us DVE reduce
     sitting in the tail's critical DVE chain.
  4. The last deferred epilogue's ACT reduces are emitted AFTER the
     final scores/exp so they fill ACT idle slots instead of delaying
     the final exp (v2 measured a 3.4us exp wait).
  5. Final ctx reduces split ACT/DVE; first ctx multiply chunked so it
     starts on the first broadcast chunk.

Carried over from v2 (all trace-validated):
  - Main loop (at, ec-outer, sc-inner), stationary reused, mm_acc 6
    PSUM banks; per-MM spacing is 216ns (LDWEIGHTS fully hidden).
  - Scores col-tiled into ONE PSUM bank: stripes at partitions
    {0,32,64,96} via tile_position, memset + start=False accumulation
    (no whole-bank has_written-clear hazard), measured concurrent
    (4 MMs per ~390ns).  Whole-bank exp in one ACT op.
  - Softmax denominator for deferred epilogues from a DVE reduce of the
    broadcast wbc (every partition computes the total).
  - Deferred epilogue split across the carrier's at-groups; epilogue
    normalize-muls on GPSIMD, stores on the scalar ring, so the DVE
    queue (PSUM-release casts) and sync queue (x loads) never block.
  - Final epilogue: PE K=1 ones-matmul broadcast from the stripe rows,
    drains ping-ponged DVE/ACT.
  - proj_q warmup transposed: ONE 8-MM N=512 accumulation (~3.5us HAM
    warmup), result bounced through DRAM into [a-part, at, b] layout.
"""

import sys

sys.path.insert(0, "/opt/trn_rl_repo")

import ml_dtypes
import numpy as np

import concourse.bass as bass
import concourse.tile as tile
from concourse import bacc, bass_utils, mybir

B, S, E, Q, A = 32, 2048, 1024, 1024, 512
NCORES = 8
BPC = B // NCORES  # batches per core
P = 128
EC = E // P  # 8 e-chunks
QC = Q // P  # 8 q-chunks
AT = A // P  # 4 a-tiles
SF = 512  # matmul moving free dim
SC = S // SF  # 4 s-chunks
QPAD = 128  # padded free dim for the transposed proj_q warmup

BF = mybir.dt.bfloat16
F32 = mybir.dt.float32
TANH = mybir.ActivationFunctionType.Tanh
EXP = mybir.ActivationFunctionType.Exp
COPY = mybir.ActivationFunctionType.Copy


def build():
    nc = bacc.Bacc("TRN2", target_bir_lowering=False, debug=False)

    xT = nc.dram_tensor("xT", [BPC, E, S], BF, kind="ExternalInput")
    qT = nc.dram_tensor("qT", [Q, QPAD], BF, kind="ExternalInput")
    w_in = nc.dram_tensor("w_in", [E, A], BF, kind="ExternalInput")
    w_q = nc.dram_tensor("w_q", [Q, A], BF, kind="ExternalInput")
    w_att = nc.dram_tensor("w_att", [A], BF, kind="ExternalInput")
    out = nc.dram_tensor("out", [BPC, A], F32, kind="ExternalOutput")

    with tile.TileContext(nc) as tc:
        with (
            tc.tile_pool(name="const", bufs=1) as const,
            tc.tile_pool(name="xtp", bufs=2) as xtp,
            tc.tile_pool(name="ttp", bufs=2) as ttp,
            tc.tile_pool(name="small", bufs=3) as small,
            tc.tile_pool(name="mm_ps", bufs=6, space="PSUM") as mm_ps,
            tc.tile_pool(name="sc_ps", bufs=2, space="PSUM") as sc_ps,
            tc.tile_pool(name="dram", bufs=2, space="DRAM") as dram,
        ):
            # ---- weights: sync-ring head (hardware DGE); w_in on scalar
            qT_sb = const.tile([P, QC, QPAD], BF)
            nc.sync.dma_start(
                qT_sb,
                bass.AP(tensor=qT, offset=0, ap=[[QPAD, P], [P * QPAD, QC], [1, QPAD]]),
            )
            wq_sb = const.tile([P, QC, A], BF)
            nc.sync.dma_start(
                wq_sb,
                bass.AP(tensor=w_q, offset=0, ap=[[A, P], [P * A, QC], [1, A]]),
            )
            watt_sb = const.tile([P, AT], BF)
            nc.sync.dma_start(watt_sb, w_att.ap().rearrange("(at p) -> p at", p=P))

            w_sb = const.tile([P, EC, AT, P], BF)
            w_in_r = bass.AP(
                tensor=w_in,
                offset=0,
                ap=[[A, P], [P * A, EC], [P, AT], [1, P]],
            )
            nc.scalar.dma_start(w_sb, w_in_r)

            ones2 = const.tile([P, P], BF)
            nc.vector.memset(ones2, 1.0)
            ones_f = const.tile([1, P], F32)
            nc.vector.memset(ones_f, 1.0)
            mask_f = const.tile([P, 1], F32)
            nc.vector.memset(mask_f, 0.0)
            for sc in range(SC):
                nc.vector.memset(mask_f[32 * sc : 32 * sc + 1, :], 1.0)

            # ---- proj_q warmup (transposed; one 8-MM N=512 chain)
            pq_ps = mm_ps.tile([P, SF], F32, name="mm_acc")
            for qc in range(QC):
                nc.tensor.matmul(
                    pq_ps,
                    qT_sb[:, qc, :],
                    wq_sb[:, qc, :],
                    start=(qc == 0),
                    stop=(qc == QC - 1),
                )
            pq_flat = small.tile([P, A], F32, name="pq_flat", bufs=1)
            nc.scalar.copy(pq_flat[:BPC, :], pq_ps[:BPC, :])
            pq_dram = dram.tile([A, BPC], F32, name="pq_dram")
            nc.scalar.dma_start(
                bass.AP(
                    tensor=pq_dram.tensor,
                    offset=pq_dram.offset,
                    ap=[[1, BPC], [BPC, A]],
                ),
                pq_flat[:BPC, :],
            )
            projq = const.tile([P, AT, BPC], F32)
            nc.scalar.dma_start(
                projq,
                bass.AP(
                    tensor=pq_dram.tensor,
                    offset=pq_dram.offset,
                    ap=[[BPC, P], [P * BPC, AT], [1, BPC]],
                ),
            )

            # ---- epilogue pieces -------------------------------------
            def emit_scores(pts):
                """Col-tiled scores: ONE PSUM bank, 4 stripes at partitions
                {0,32,64,96}; start=False onto a zeroed bank; whole-bank exp
                in one ACT op."""
                sps = sc_ps.tile([P, SF], F32, name="sps")
                nc.vector.memset(sps, 0.0)
                for at in range(AT):
                    for sc in range(SC):
                        nc.tensor.matmul(
                            sps[32 * sc : 32 * sc + 1, :],
                            watt_sb[:, at : at + 1],
                            pts[at][:, sc * SF : (sc + 1) * SF],
                            start=False,
                            stop=(at == AT - 1),
                            skip_group_check=True,
                            tile_position=(0, 32 * sc),
                        )
                exp_sb = small.tile([P, SF], BF, name="exp_sb")
                esum = small.tile([P, 1], F32, name="esum")
                nc.scalar.activation(exp_sb, sps, EXP, accum_out=esum)
                return exp_sb, esum

            def emit_wbc_dma(exp_sb):
                # gather the 4 stripe rows into DRAM, broadcast back to all
                # 128 partitions (stride-0 read); sync ring (shared with x
                # loads - measured fine in v2)
                exp_dram = dram.tile([1, S], BF, name="exp_dram")
                nc.sync.dma_start(
                    bass.AP(
                        tensor=exp_dram.tensor,
                        offset=exp_dram.offset,
                        ap=[[SF, SC], [1, SF]],
                    ),
                    exp_sb[0 : 32 * SC - 31 : 32, :],
                )
                wbc = ttp.tile([P, S], BF, name="wbc")
                nc.sync.dma_start(
                    wbc,
                    bass.AP(
                        tensor=exp_dram.tensor,
                        offset=exp_dram.offset,
                        ap=[[0, P], [1, S]],
                    ),
                )
                return wbc

            def emit_tot_rcp(wbc):
                tot = small.tile([P, 1], F32, name="tot")
                nc.vector.tensor_reduce(
                    tot, wbc, axis=mybir.AxisListType.X, op=mybir.AluOpType.add
                )
                rcp = small.tile([P, 1], F32, name="rcp")
                nc.vector.reciprocal(rcp, tot)
                return rcp

            # ---- main batch loop -------------------------------------
            prev = None  # (batch_idx, t tiles, projTall)
            ep = {}  # in-flight deferred epilogue state
            for b in range(BPC):
                last = b == BPC - 1
                xpairs = []
                for h in range(EC // 2):
                    xp = xtp.tile([P, 2, S], BF, name=f"xp{h}")
                    nc.sync.dma_start(
                        xp,
                        bass.AP(
                            tensor=xT,
                            offset=(b * E + h * 2 * P) * S,
                            ap=[[S, P], [P * S, 2], [1, S]],
                        ),
                    )
                    xpairs.append(xp)

                ts_ = []
                projTall = ttp.tile([P, AT * S], BF, name="projTall", bufs=3)
                for at in range(AT):
                    t_sb = ttp.tile([P, S], BF, name=f"t{at}")
                    pss = [mm_ps.tile([P, SF], F32, name="mm_acc") for _ in range(SC)]
                    for ec in range(EC):
                        for sc in range(SC):
                            nc.tensor.matmul(
                                pss[sc],
                                w_sb[:, ec, at, :],
                                xpairs[ec // 2][:, ec % 2, sc * SF : (sc + 1) * SF],
                                start=(ec == 0),
                                stop=(ec == EC - 1),
                            )
                    for sc in range(SC):
                        sl = slice(at * S + sc * SF, at * S + (sc + 1) * SF)
                        # single PSUM reader (DVE cast) gates PSUM release;
                        # tanh reads the SBUF copy with the proj_q bias fused
                        nc.vector.tensor_copy(projTall[:, sl], pss[sc])
                        nc.scalar.activation(
                            t_sb[:, sc * SF : (sc + 1) * SF],
                            projTall[:, sl],
                            TANH,
                            bias=projq[:, at, b : b + 1],
                        )
                    ts_.append(t_sb)

                    if at == 1 and prev is not None:
                        # deferred epilogue part 1: scores / exp / broadcast
                        ep["b"], ep["ts"], ep["proj"] = prev
                        ep["exp"], _ = emit_scores(ep["ts"])
                        ep["wbc"] = emit_wbc_dma(ep["exp"])
                        ep["cscr"] = [
                            ttp.tile([P, S], BF, name=f"cscr{i}", bufs=1)
                            for i in range(AT)
                        ]
                    if at == 2 and prev is not None:
                        # part 2a: first two ctx multiplies
                        for i in range(2):
                            nc.vector.tensor_tensor(
                                out=ep["cscr"][i],
                                in0=ep["proj"][:, i * S : (i + 1) * S],
                                in1=ep["wbc"],
                                op=mybir.AluOpType.mult,
                            )

                # part 2b: remaining mults + denominator (DVE); for b0/b1
                # carriers also the reduces/store; b2's reduces move past
                # the final scores/exp (delta 4)
                if prev is not None:
                    for i in range(2, AT):
                        nc.vector.tensor_tensor(
                            out=ep["cscr"][i],
                            in0=ep["proj"][:, i * S : (i + 1) * S],
                            in1=ep["wbc"],
                            op=mybir.AluOpType.mult,
                        )
                    ep["rcp"] = emit_tot_rcp(ep["wbc"])
                    ep["c"] = small.tile([P, AT], F32, name="c")
                    if not last:
                        for i in range(AT):
                            nc.scalar.activation(
                                ep["cscr"][i], ep["cscr"][i], COPY,
                                accum_out=ep["c"][:, i : i + 1],
                            )
                        for i in range(AT):
                            nc.gpsimd.tensor_scalar_mul(
                                ep["c"][:, i : i + 1], ep["c"][:, i : i + 1], ep["rcp"]
                            )
                        nc.scalar.dma_start(
                            bass.AP(
                                tensor=out, offset=ep["b"] * A, ap=[[1, P], [P, AT]]
                            ),
                            ep["c"],
                        )

                prev = (b, ts_, projTall)

            # ---- final epilogue (latency-critical, PE idle afterwards) --
            pb, pts, pproj = prev
            exp_sb, esum = emit_scores(pts)
            # denominator via tiny PE matmuls: tot = mask . esum, broadcast
            # with a K=1 ones matmul, reciprocal on [128,1] (delta 3)
            tot_ps = mm_ps.tile([P, SF], F32, name="mm_acc")
            nc.tensor.matmul(tot_ps[:1, :1], mask_f, esum, start=True, stop=True)
            tot_sb = small.tile([1, 1], F32, name="tot_sb")
            nc.scalar.copy(tot_sb, tot_ps[:1, :1])
            totbc_ps = mm_ps.tile([P, SF], F32, name="mm_acc")
            nc.tensor.matmul(totbc_ps[:, :1], ones_f, tot_sb, start=True, stop=True)
            # PE K=1 ones-matmul broadcast of the exp stripe rows
            wbc = ttp.tile([P, S], BF, name="wbc")
            wpss = []
            for sc in range(SC):
                wps = mm_ps.tile([P, SF], F32, name="mm_acc")
                nc.tensor.matmul(
                    wps,
                    ones2[32 * sc : 32 * sc + 1, :],
                    exp_sb[32 * sc : 32 * sc + 1, :],
                    start=True,
                    stop=True,
                    tile_position=(32 * sc, 0),
                )
                wpss.append(wps)
            for sc in range(SC):
                dst = wbc[:, sc * SF : (sc + 1) * SF]
                if sc % 2 == 0:
                    nc.vector.tensor_copy(dst, wpss[sc])
                else:
                    nc.scalar.copy(dst, wpss[sc])
            totbc = small.tile([P, 1], F32, name="totbc")
            nc.vector.tensor_copy(totbc, totbc_ps[:, :1])
            rcp = small.tile([P, 1], F32, name="rcp")
            nc.vector.reciprocal(rcp, totbc)

            # b2's reduces/normalize/store now fill the ACT idle slots
            # behind the final exp (delta 4)
            for i in range(2):
                nc.scalar.activation(
                    ep["cscr"][i], ep["cscr"][i], COPY, accum_out=ep["c"][:, i : i + 1]
                )

            # final ctx: first multiply chunked so it starts on the first
            # broadcast chunk; reduces split ACT/DVE (delta 5)
            cscrs = [ttp.tile([P, S], BF, name=f"cscr{i}", bufs=1) for i in range(AT)]
            c = small.tile([P, AT], F32, name="c")
            for sc in range(SC):
                sl = slice(sc * SF, (sc + 1) * SF)
                nc.vector.tensor_tensor(
                    out=cscrs[0][:, sl],
                    in0=pproj[:, sc * SF : (sc + 1) * SF],
                    in1=wbc[:, sl],
                    op=mybir.AluOpType.mult,
                )
            for i in (1, 2, 3):
                nc.vector.tensor_tensor(
                    out=cscrs[i],
                    in0=pproj[:, i * S : (i + 1) * S],
                    in1=wbc,
                    op=mybir.AluOpType.mult,
                )
            for i in range(2, AT):
                nc.scalar.activation(
                    ep["cscr"][i], ep["cscr"][i], COPY, accum_out=ep["c"][:, i : i + 1]
                )
            for i in range(AT):
                nc.gpsimd.tensor_scalar_mul(
                    ep["c"][:, i : i + 1], ep["c"][:, i : i + 1], ep["rcp"]
                )
            nc.scalar.dma_start(
                bass.AP(tensor=out, offset=ep["b"] * A, ap=[[1, P], [P, AT]]),
                ep["c"],
            )
            nc.scalar.activation(cscrs[0], cscrs[0], COPY, accum_out=c[:, 0:1])
            nc.vector.tensor_reduce(
                c[:, 1:2], cscrs[1], axis=mybir.AxisListType.X, op=mybir.AluOpType.add
            )
            nc.scalar.activation(cscrs[2], cscrs[2], COPY, accum_out=c[:, 2:3])
            nc.vector.tensor_reduce(
                c[:, 3:4], cscrs[3], axis=mybir.AxisListType.X, op=mybir.AluOpType.add
            )
            for at in range(AT):
                nc.vector.tensor_scalar_mul(c[:, at : at + 1], c[:, at : at + 1], rcp)
            nc.sync.dma_start(
                bass.AP(tensor=out, offset=pb * A, ap=[[1, P], [P, AT]]),
                c,
            )

    nc.compile()
    return nc


_nc = None


def kernel(inputs, query, W_in, W_q, w_att):
    global _nc
    if _nc is None:
        _nc = build()

    bf = ml_dtypes.bfloat16
    x_bf = np.asarray(inputs).astype(bf)
    xT_bf = np.ascontiguousarray(x_bf.transpose(0, 2, 1))
    w_in_bf = np.ascontiguousarray(np.asarray(W_in).astype(bf))
    w_q_bf = np.ascontiguousarray(np.asarray(W_q).astype(bf))
    w_att_bf = np.ascontiguousarray(np.asarray(w_att).astype(bf))

    in_maps = []
    for c in range(NCORES):
        sl = slice(c * BPC, (c + 1) * BPC)
        qTp = np.zeros((Q, QPAD), dtype=bf)
        qTp[:, :BPC] = np.asarray(query[sl]).astype(bf).T
        in_maps.append(
            {
                "xT": np.ascontiguousarray(xT_bf[sl]),
                "qT": qTp,
                "w_in": w_in_bf,
                "w_q": w_q_bf,
                "w_att": w_att_bf,
            }
        )

    res = bass_utils.run_bass_kernel_spmd(_nc, in_maps, core_ids=list(range(NCORES)))
    return np.concatenate([r["out"] for r in res.results], axis=0)


if __name__ == "__main__":
    rng = np.random.default_rng(0)
    ins = {
        "inputs": rng.standard_normal((B, S, E), dtype=np.float32),
        "query": rng.standard_normal((B, Q), dtype=np.float32),
        "W_in": (rng.standard_normal((E, A), dtype=np.float32) / np.sqrt(E)).astype(
            np.float32
        ),
        "W_q": (rng.standard_normal((Q, A), dtype=np.float32) / np.sqrt(Q)).astype(
            np.float32
        ),
        "w_att": (rng.standard_normal((A,), dtype=np.float32) / np.sqrt(A)).astype(
            np.float32
        ),
    }
    got = kernel(**ins)
    print("out shape", got.shape, got.dtype)


# revision 12
# speedup vs baseline: 1.3165x; 1.1252x over previous
"""Bahdanau (additive) attention TRN2 Bass kernel (v5).

reference:
    proj_in = einsum("bse,ea->bsa", inputs, W_in)      # [B,S,A]
    proj_q  = (query @ W_q)[:, None, :]                # [B,1,A]
    scores  = einsum("bsa,a->bs", tanh(proj_in+proj_q), w_att)
    weights = softmax(scores, axis=1)
    context = einsum("bs,bsa->ba", weights, proj_in)   # [B,A]

B,S,E,Q,A = 32,2048,1024,1024,512.

Sharding: data-parallel over batch. 8 cores x 4 batches each; weights
replicated. No collectives; host scatters inputs / gathers outputs.

v5 = the measured-best v2 structure (178.3us) + five local deltas:
  1. qT/wq/watt loads moved from the gpsimd ring (software DGE - they
     completed at ~17us and stalled the warmup) to the HEAD of the sync
     hardware ring, before the x loads.  w_in stays on the scalar ring.
  2. x loaded as pair-DMAs [128,2,S]: 16 instead of 32 DMA instructions
     (the end-of-kernel drain serially retires ~16 completion pokes per
     DMA instruction at ~20ns).
  3. Final softmax denominator via tiny PE matmuls (mask-dot of the exp
     accumulator + K=1 ones broadcast) instead of a 2.27us DVE reduce
     sitting in the tail's critical DVE chain.
  4. The last deferred epilogue's ACT reduces are emitted AFTER the
     final scores/exp so they fill ACT idle slots instead of delaying
     the final exp (v2 measured a 3.4us exp wait).
  5. Final ctx reduces split ACT/DVE; first ctx multiply chunked so it
     starts on the first broadcast chunk.

Carried over from v2 (all trace-validated):
  - Main loop (at, ec-outer, sc-inner), stationary reused, mm_acc 6
    PSUM banks; per-MM spacing is 216ns (LDWEIGHTS fully hidden).
  - Scores col-tiled into ONE PSUM bank: stripes at partitions
    {0,32,64,96} via tile_position, memset + start=False accumulation
    (no whole-bank has_written-clear hazard), measured concurrent
    (4 MMs per ~390ns).  Whole-bank exp in one ACT op.
  - Softmax denominator for deferred epilogues from a DVE reduce of the
    broadcast wbc (every partition computes the total).
  - Deferred epilogue split across the carrier's at-groups; epilogue
    normalize-muls on GPSIMD, stores on the scalar ring, so the DVE
    queue (PSUM-release casts) and sync queue (x loads) never block.
  - Final epilogue: PE K=1 ones-matmul broadcast from the stripe rows,
    drains ping-ponged DVE/ACT.
  - proj_q warmup transposed: ONE 8-MM N=512 accumulation (~3.5us HAM
    warmup), result bounced through DRAM into [a-part, at, b] layout.
"""

import sys

sys.path.insert(0, "/opt/trn_rl_repo")

import ml_dtypes
import numpy as np

import concourse.bass as bass
import concourse.tile as tile
from concourse import bacc, bass_utils, mybir

B, S, E, Q, A = 32, 2048, 1024, 1024, 512
NCORES = 8
BPC = B // NCORES  # batches per core
P = 128
EC = E // P  # 8 e-chunks
QC = Q // P  # 8 q-chunks
AT = A // P  # 4 a-tiles
SF = 512  # matmul moving free dim
SC = S // SF  # 4 s-chunks
QPAD = 128  # padded free dim for the transposed proj_q warmup

BF = mybir.dt.bfloat16
F32 = mybir.dt.float32
TANH = mybir.ActivationFunctionType.Tanh
EXP = mybir.ActivationFunctionType.Exp
COPY = mybir.ActivationFunctionType.Copy


def build():
    nc = bacc.Bacc("TRN2", target_bir_lowering=False, debug=False)

    xT = nc.dram_tensor("xT", [BPC, E, S], BF, kind="ExternalInput")
    qT = nc.dram_tensor("qT", [Q, QPAD], BF, kind="ExternalInput")
    w_in = nc.dram_tensor("w_in", [E, A], BF, kind="ExternalInput")
    w_q = nc.dram_tensor("w_q", [Q, A], BF, kind="ExternalInput")
    w_att = nc.dram_tensor("w_att", [A], BF, kind="ExternalInput")
    out = nc.dram_tensor("out", [BPC, A], F32, kind="ExternalOutput")

    with tile.TileContext(nc) as tc:
        with (
            tc.tile_pool(name="const", bufs=1) as const,
            tc.tile_pool(name="xtp", bufs=2) as xtp,
            tc.tile_pool(name="ttp", bufs=2) as ttp,
            tc.tile_pool(name="small", bufs=3) as small,
            tc.tile_pool(name="mm_ps", bufs=6, space="PSUM") as mm_ps,
            tc.tile_pool(name="sc_ps", bufs=2, space="PSUM") as sc_ps,
            tc.tile_pool(name="dram", bufs=2, space="DRAM") as dram,
        ):
            # ---- weights: sync-ring head (hardware DGE); w_in on scalar
            qT_sb = const.tile([P, QC, QPAD], BF)
            nc.sync.dma_start(
                qT_sb,
                bass.AP(tensor=qT, offset=0, ap=[[QPAD, P], [P * QPAD, QC], [1, QPAD]]),
            )
            wq_sb = const.tile([P, QC, A], BF)
            nc.sync.dma_start(
                wq_sb,
                bass.AP(tensor=w_q, offset=0, ap=[[A, P], [P * A, QC], [1, A]]),
            )
            watt_sb = const.tile([P, AT], BF)
            nc.sync.dma_start(watt_sb, w_att.ap().rearrange("(at p) -> p at", p=P))

            w_sb = const.tile([P, EC, AT, P], BF)
            w_in_r = bass.AP(
                tensor=w_in,
                offset=0,
                ap=[[A, P], [P * A, EC], [P, AT], [1, P]],
            )
            nc.scalar.dma_start(w_sb, w_in_r)

            ones2 = const.tile([P, P], BF)
            nc.vector.memset(ones2, 1.0)
            ones_f = const.tile([1, P], F32)
            nc.vector.memset(ones_f, 1.0)
            mask_f = const.tile([P, 1], F32)
            nc.vector.memset(mask_f, 0.0)
            for sc in range(SC):
                nc.vector.memset(mask_f[32 * sc : 32 * sc + 1, :], 1.0)

            # ---- proj_q warmup, baseline-style (no DRAM bounce - a
            # bounced bias acquired spurious queue dependencies and
            # starved every tanh): per-at accumulation with wq stationary,
            # qT moving; PSUM copied straight to the per-at bias tiles.
            # 32 N=128 MMs also serve as the HAM warmup.
            projq = const.tile([P, AT, BPC], F32)
            for at in range(AT):
                pq_ps = mm_ps.tile([P, SF], F32, name="mm_acc")
                for qc in range(QC):
                    nc.tensor.matmul(
                        pq_ps[:, :QPAD],
                        wq_sb[:, qc, at * P : (at + 1) * P],
                        qT_sb[:, qc, :],
                        start=(qc == 0),
                        stop=(qc == QC - 1),
                    )
                nc.scalar.copy(projq[:, at, :], pq_ps[:, :BPC])

            # ---- epilogue pieces -------------------------------------
            def emit_scores(pts):
                """Col-tiled scores: ONE PSUM bank, 4 stripes at partitions
                {0,32,64,96}; start=False onto a zeroed bank; whole-bank exp
                in one ACT op."""
                sps = sc_ps.tile([P, SF], F32, name="sps")
                nc.vector.memset(sps, 0.0)
                for at in range(AT):
                    for sc in range(SC):
                        nc.tensor.matmul(
                            sps[32 * sc : 32 * sc + 1, :],
                            watt_sb[:, at : at + 1],
                            pts[at][:, sc * SF : (sc + 1) * SF],
                            start=False,
                            stop=(at == AT - 1),
                            skip_group_check=True,
                            tile_position=(0, 32 * sc),
                        )
                exp_sb = small.tile([P, SF], BF, name="exp_sb")
                esum = small.tile([P, 1], F32, name="esum")
                nc.scalar.activation(exp_sb, sps, EXP, accum_out=esum)
                return exp_sb, esum

            def emit_wbc_dma(exp_sb):
                # gather the 4 stripe rows into DRAM, broadcast back to all
                # 128 partitions (stride-0 read); sync ring (shared with x
                # loads - measured fine in v2)
                exp_dram = dram.tile([1, S], BF, name="exp_dram")
                nc.sync.dma_start(
                    bass.AP(
                        tensor=exp_dram.tensor,
                        offset=exp_dram.offset,
                        ap=[[SF, SC], [1, SF]],
                    ),
                    exp_sb[0 : 32 * SC - 31 : 32, :],
                )
                wbc = ttp.tile([P, S], BF, name="wbc")
                nc.sync.dma_start(
                    wbc,
                    bass.AP(
                        tensor=exp_dram.tensor,
                        offset=exp_dram.offset,
                        ap=[[0, P], [1, S]],
                    ),
                )
                return wbc

            def emit_tot_rcp(wbc):
                tot = small.tile([P, 1], F32, name="tot")
                nc.vector.tensor_reduce(
                    tot, wbc, axis=mybir.AxisListType.X, op=mybir.AluOpType.add
                )
                rcp = small.tile([P, 1], F32, name="rcp")
                nc.vector.reciprocal(rcp, tot)
                return rcp

            # ---- main batch loop -------------------------------------
            prev = None  # (batch_idx, t tiles, projTall)
            ep = {}  # in-flight deferred epilogue state
            for b in range(BPC):
                last = b == BPC - 1
                xpairs = []
                for h in range(EC // 2):
                    xp = xtp.tile([P, 2, S], BF, name=f"xp{h}")
                    nc.sync.dma_start(
                        xp,
                        bass.AP(
                            tensor=xT,
                            offset=(b * E + h * 2 * P) * S,
                            ap=[[S, P], [P * S, 2], [1, S]],
                        ),
                    )
                    xpairs.append(xp)

                ts_ = []
                projTall = ttp.tile([P, AT * S], BF, name="projTall", bufs=3)
                for at in range(AT):
                    t_sb = ttp.tile([P, S], BF, name=f"t{at}")
                    pss = [mm_ps.tile([P, SF], F32, name="mm_acc") for _ in range(SC)]
                    for ec in range(EC):
                        for sc in range(SC):
                            nc.tensor.matmul(
                                pss[sc],
                                w_sb[:, ec, at, :],
                                xpairs[ec // 2][:, ec % 2, sc * SF : (sc + 1) * SF],
                                start=(ec == 0),
                                stop=(ec == EC - 1),
                            )
                    for sc in range(SC):
                        sl = slice(at * S + sc * SF, at * S + (sc + 1) * SF)
                        # single PSUM reader (DVE cast) gates PSUM release;
                        # tanh reads the SBUF copy with the proj_q bias fused
                        nc.vector.tensor_copy(projTall[:, sl], pss[sc])
                        nc.scalar.activation(
                            t_sb[:, sc * SF : (sc + 1) * SF],
                            projTall[:, sl],
                            TANH,
                            bias=projq[:, at, b : b + 1],
                        )
                    ts_.append(t_sb)

                    if at == 1 and prev is not None:
                        # deferred epilogue part 1: scores / exp / broadcast
                        ep["b"], ep["ts"], ep["proj"] = prev
                        ep["exp"], _ = emit_scores(ep["ts"])
                        ep["wbc"] = emit_wbc_dma(ep["exp"])
                        ep["cscr"] = [
                            ttp.tile([P, S], BF, name=f"cscr{i}", bufs=1)
                            for i in range(AT)
                        ]
                    if at == 2 and prev is not None:
                        # part 2a: first two ctx multiplies
                        for i in range(2):
                            nc.vector.tensor_tensor(
                                out=ep["cscr"][i],
                                in0=ep["proj"][:, i * S : (i + 1) * S],
                                in1=ep["wbc"],
                                op=mybir.AluOpType.mult,
                            )

                # part 2b: remaining mults + denominator (DVE); for b0/b1
                # carriers also the reduces/store; b2's reduces move past
                # the final scores/exp (delta 4)
                if prev is not None:
                    for i in range(2, AT):
                        nc.vector.tensor_tensor(
                            out=ep["cscr"][i],
                            in0=ep["proj"][:, i * S : (i + 1) * S],
                            in1=ep["wbc"],
                            op=mybir.AluOpType.mult,
                        )
                    ep["rcp"] = emit_tot_rcp(ep["wbc"])
                    ep["c"] = small.tile([P, AT], F32, name="c")
                    if not last:
                        for i in range(AT):
                            nc.scalar.activation(
                                ep["cscr"][i], ep["cscr"][i], COPY,
                                accum_out=ep["c"][:, i : i + 1],
                            )
                        for i in range(AT):
                            nc.gpsimd.tensor_scalar_mul(
                                ep["c"][:, i : i + 1], ep["c"][:, i : i + 1], ep["rcp"]
                            )
                        nc.scalar.dma_start(
                            bass.AP(
                                tensor=out, offset=ep["b"] * A, ap=[[1, P], [P, AT]]
                            ),
                            ep["c"],
                        )

                prev = (b, ts_, projTall)

            # ---- final epilogue (latency-critical, PE idle afterwards) --
            pb, pts, pproj = prev
            exp_sb, esum = emit_scores(pts)
            # denominator via tiny PE matmuls: tot = mask . esum, broadcast
            # with a K=1 ones matmul, reciprocal on [128,1] (delta 3)
            tot_ps = mm_ps.tile([P, SF], F32, name="mm_acc")
            nc.tensor.matmul(tot_ps[:1, :1], mask_f, esum, start=True, stop=True)
            tot_sb = small.tile([1, 1], F32, name="tot_sb")
            nc.scalar.copy(tot_sb, tot_ps[:1, :1])
            totbc_ps = mm_ps.tile([P, SF], F32, name="mm_acc")
            nc.tensor.matmul(totbc_ps[:, :1], ones_f, tot_sb, start=True, stop=True)
            # PE K=1 ones-matmul broadcast of the exp stripe rows
            wbc = ttp.tile([P, S], BF, name="wbc")
            wpss = []
            for sc in range(SC):
                wps = mm_ps.tile([P, SF], F32, name="mm_acc")
                nc.tensor.matmul(
                    wps,
                    ones2[32 * sc : 32 * sc + 1, :],
                    exp_sb[32 * sc : 32 * sc + 1, :],
                    start=True,
                    stop=True,
                    tile_position=(32 * sc, 0),
                )
                wpss.append(wps)
            for sc in range(SC):
                dst = wbc[:, sc * SF : (sc + 1) * SF]
                if sc % 2 == 0:
                    nc.vector.tensor_copy(dst, wpss[sc])
                else:
                    nc.scalar.copy(dst, wpss[sc])
            totbc = small.tile([P, 1], F32, name="totbc")
            nc.vector.tensor_copy(totbc, totbc_ps[:, :1])
            rcp = small.tile([P, 1], F32, name="rcp")
            nc.vector.reciprocal(rcp, totbc)

            # b2's reduces/normalize/store now fill the ACT idle slots
            # behind the final exp (delta 4)
            for i in range(2):
                nc.scalar.activation(
                    ep["cscr"][i], ep["cscr"][i], COPY, accum_out=ep["c"][:, i : i + 1]
                )

            # final ctx: first multiply chunked so it starts on the first
            # broadcast chunk; reduces split ACT/DVE (delta 5)
            cscrs = [ttp.tile([P, S], BF, name=f"cscr{i}", bufs=1) for i in range(AT)]
            c = small.tile([P, AT], F32, name="c")
            for sc in range(SC):
                sl = slice(sc * SF, (sc + 1) * SF)
                nc.vector.tensor_tensor(
                    out=cscrs[0][:, sl],
                    in0=pproj[:, sc * SF : (sc + 1) * SF],
                    in1=wbc[:, sl],
                    op=mybir.AluOpType.mult,
                )
            for i in (1, 2, 3):
                nc.vector.tensor_tensor(
                    out=cscrs[i],
                    in0=pproj[:, i * S : (i + 1) * S],
                    in1=wbc,
                    op=mybir.AluOpType.mult,
                )
            for i in range(2, AT):
                nc.scalar.activation(
                    ep["cscr"][i], ep["cscr"][i], COPY, accum_out=ep["c"][:, i : i + 1]
                )
            for i in range(AT):
                nc.gpsimd.tensor_scalar_mul(
                    ep["c"][:, i : i + 1], ep["c"][:, i : i + 1], ep["rcp"]
                )
            nc.scalar.dma_start(
                bass.AP(tensor=out, offset=ep["b"] * A, ap=[[1, P], [P, AT]]),
                ep["c"],
            )
            nc.scalar.activation(cscrs[0], cscrs[0], COPY, accum_out=c[:, 0:1])
            nc.vector.tensor_reduce(
                c[:, 1:2], cscrs[1], axis=mybir.AxisListType.X, op=mybir.AluOpType.add
            )
            nc.scalar.activation(cscrs[2], cscrs[2], COPY, accum_out=c[:, 2:3])
            nc.vector.tensor_reduce(
                c[:, 3:4], cscrs[3], axis=mybir.AxisListType.X, op=mybir.AluOpType.add
            )
            for at in range(AT):
                nc.vector.tensor_scalar_mul(c[:, at : at + 1], c[:, at : at + 1], rcp)
            nc.sync.dma_start(
                bass.AP(tensor=out, offset=pb * A, ap=[[1, P], [P, AT]]),
                c,
            )

    nc.compile()
    return nc


_nc = None


def kernel(inputs, query, W_in, W_q, w_att):
    global _nc
    if _nc is None:
        _nc = build()

    bf = ml_dtypes.bfloat16
    x_bf = np.asarray(inputs).astype(bf)
    xT_bf = np.ascontiguousarray(x_bf.transpose(0, 2, 1))
    w_in_bf = np.ascontiguousarray(np.asarray(W_in).astype(bf))
    w_q_bf = np.ascontiguousarray(np.asarray(W_q).astype(bf))
    w_att_bf = np.ascontiguousarray(np.asarray(w_att).astype(bf))

    in_maps = []
    for c in range(NCORES):
        sl = slice(c * BPC, (c + 1) * BPC)
        qTp = np.zeros((Q, QPAD), dtype=bf)
        qTp[:, :BPC] = np.asarray(query[sl]).astype(bf).T
        in_maps.append(
            {
                "xT": np.ascontiguousarray(xT_bf[sl]),
                "qT": qTp,
                "w_in": w_in_bf,
                "w_q": w_q_bf,
                "w_att": w_att_bf,
            }
        )

    res = bass_utils.run_bass_kernel_spmd(_nc, in_maps, core_ids=list(range(NCORES)))
    return np.concatenate([r["out"] for r in res.results], axis=0)


if __name__ == "__main__":
    rng = np.random.default_rng(0)
    ins = {
        "inputs": rng.standard_normal((B, S, E), dtype=np.float32),
        "query": rng.standard_normal((B, Q), dtype=np.float32),
        "W_in": (rng.standard_normal((E, A), dtype=np.float32) / np.sqrt(E)).astype(
            np.float32
        ),
        "W_q": (rng.standard_normal((Q, A), dtype=np.float32) / np.sqrt(Q)).astype(
            np.float32
        ),
        "w_att": (rng.standard_normal((A,), dtype=np.float32) / np.sqrt(A)).astype(
            np.float32
        ),
    }
    got = kernel(**ins)
    print("out shape", got.shape, got.dtype)


# revision 16
# speedup vs baseline: 1.3241x; 1.0057x over previous
"""Bahdanau (additive) attention TRN2 Bass kernel (v5).

reference:
    proj_in = einsum("bse,ea->bsa", inputs, W_in)      # [B,S,A]
    proj_q  = (query @ W_q)[:, None, :]                # [B,1,A]
    scores  = einsum("bsa,a->bs", tanh(proj_in+proj_q), w_att)
    weights = softmax(scores, axis=1)
    context = einsum("bs,bsa->ba", weights, proj_in)   # [B,A]

B,S,E,Q,A = 32,2048,1024,1024,512.

Sharding: data-parallel over batch. 8 cores x 4 batches each; weights
replicated. No collectives; host scatters inputs / gathers outputs.

v5 = the measured-best v2 structure (178.3us) + five local deltas:
  1. qT/wq/watt loads moved from the gpsimd ring (software DGE - they
     completed at ~17us and stalled the warmup) to the HEAD of the sync
     hardware ring, before the x loads.  w_in stays on the scalar ring.
  2. x loaded as pair-DMAs [128,2,S]: 16 instead of 32 DMA instructions
     (the end-of-kernel drain serially retires ~16 completion pokes per
     DMA instruction at ~20ns).
  3. Final softmax denominator via tiny PE matmuls (mask-dot of the exp
     accumulator + K=1 ones broadcast) instead of a 2.27us DVE reduce
     sitting in the tail's critical DVE chain.
  4. The last deferred epilogue's ACT reduces are emitted AFTER the
     final scores/exp so they fill ACT idle slots instead of delaying
     the final exp (v2 measured a 3.4us exp wait).
  5. Final ctx reduces split ACT/DVE; first ctx multiply chunked so it
     starts on the first broadcast chunk.

Carried over from v2 (all trace-validated):
  - Main loop (at, ec-outer, sc-inner), stationary reused, mm_acc 6
    PSUM banks; per-MM spacing is 216ns (LDWEIGHTS fully hidden).
  - Scores col-tiled into ONE PSUM bank: stripes at partitions
    {0,32,64,96} via tile_position, memset + start=False accumulation
    (no whole-bank has_written-clear hazard), measured concurrent
    (4 MMs per ~390ns).  Whole-bank exp in one ACT op.
  - Softmax denominator for deferred epilogues from a DVE reduce of the
    broadcast wbc (every partition computes the total).
  - Deferred epilogue split across the carrier's at-groups; epilogue
    normalize-muls on GPSIMD, stores on the scalar ring, so the DVE
    queue (PSUM-release casts) and sync queue (x loads) never block.
  - Final epilogue: PE K=1 ones-matmul broadcast from the stripe rows,
    drains ping-ponged DVE/ACT.
  - proj_q warmup transposed: ONE 8-MM N=512 accumulation (~3.5us HAM
    warmup), result bounced through DRAM into [a-part, at, b] layout.
"""

import sys

sys.path.insert(0, "/opt/trn_rl_repo")

import ml_dtypes
import numpy as np

import concourse.bass as bass
import concourse.tile as tile
from concourse import bacc, bass_utils, mybir

B, S, E, Q, A = 32, 2048, 1024, 1024, 512
NCORES = 8
BPC = B // NCORES  # batches per core
P = 128
EC = E // P  # 8 e-chunks
QC = Q // P  # 8 q-chunks
AT = A // P  # 4 a-tiles
SF = 512  # matmul moving free dim
SC = S // SF  # 4 s-chunks
QPAD = 128  # padded free dim for the transposed proj_q warmup

BF = mybir.dt.bfloat16
F32 = mybir.dt.float32
TANH = mybir.ActivationFunctionType.Tanh
EXP = mybir.ActivationFunctionType.Exp
COPY = mybir.ActivationFunctionType.Copy


def build():
    nc = bacc.Bacc("TRN2", target_bir_lowering=False, debug=False)

    xT = nc.dram_tensor("xT", [BPC, E, S], BF, kind="ExternalInput")
    qT = nc.dram_tensor("qT", [Q, QPAD], BF, kind="ExternalInput")
    w_in = nc.dram_tensor("w_in", [E, A], BF, kind="ExternalInput")
    w_q = nc.dram_tensor("w_q", [Q, A], BF, kind="ExternalInput")
    w_att = nc.dram_tensor("w_att", [A], BF, kind="ExternalInput")
    out = nc.dram_tensor("out", [BPC, A], F32, kind="ExternalOutput")

    with tile.TileContext(nc) as tc:
        with (
            tc.tile_pool(name="const", bufs=1) as const,
            tc.tile_pool(name="xtp", bufs=2) as xtp,
            tc.tile_pool(name="ttp", bufs=2) as ttp,
            tc.tile_pool(name="small", bufs=3) as small,
            tc.tile_pool(name="mm_ps", bufs=6, space="PSUM") as mm_ps,
            tc.tile_pool(name="sc_ps", bufs=2, space="PSUM") as sc_ps,
            tc.tile_pool(name="dram", bufs=2, space="DRAM") as dram,
        ):
            # ---- weights: sync-ring head (hardware DGE); w_in on scalar.
            # qT/wq chunked 2 q-chunks per DMA so the warmup matmuls start
            # as soon as the first chunks land (~2.5us) instead of waiting
            # for the full 1.25MB.
            qT_sb = const.tile([P, QC, QPAD], BF)
            wq_sb = const.tile([P, QC, A], BF)
            for g in range(0, QC, 2):
                nc.sync.dma_start(
                    qT_sb[:, g : g + 2, :],
                    bass.AP(
                        tensor=qT,
                        offset=g * P * QPAD,
                        ap=[[QPAD, P], [P * QPAD, 2], [1, QPAD]],
                    ),
                )
                nc.sync.dma_start(
                    wq_sb[:, g : g + 2, :],
                    bass.AP(
                        tensor=w_q,
                        offset=g * P * A,
                        ap=[[A, P], [P * A, 2], [1, A]],
                    ),
                )
            watt_sb = const.tile([P, AT], BF)
            nc.sync.dma_start(watt_sb, w_att.ap().rearrange("(at p) -> p at", p=P))

            w_sb = const.tile([P, EC, AT, P], BF)
            w_in_r = bass.AP(
                tensor=w_in,
                offset=0,
                ap=[[A, P], [P * A, EC], [P, AT], [1, P]],
            )
            nc.scalar.dma_start(w_sb, w_in_r)

            ones2 = const.tile([P, P], BF)
            nc.vector.memset(ones2, 1.0)
            ones_f = const.tile([1, P], F32)
            nc.vector.memset(ones_f, 1.0)
            mask_f = const.tile([P, 1], F32)
            nc.vector.memset(mask_f, 0.0)
            for sc in range(SC):
                nc.vector.memset(mask_f[32 * sc : 32 * sc + 1, :], 1.0)

            # ---- proj_q warmup, baseline-style (no DRAM bounce - a
            # bounced bias acquired spurious queue dependencies and
            # starved every tanh): per-at accumulation with wq stationary,
            # qT moving; PSUM copied straight to the per-at bias tiles.
            # 32 N=128 MMs also serve as the HAM warmup.
            projq = const.tile([P, AT, BPC], F32)
            for at in range(AT):
                pq_ps = mm_ps.tile([P, SF], F32, name="mm_acc")
                for qc in range(QC):
                    nc.tensor.matmul(
                        pq_ps[:, :QPAD],
                        wq_sb[:, qc, at * P : (at + 1) * P],
                        qT_sb[:, qc, :],
                        start=(qc == 0),
                        stop=(qc == QC - 1),
                    )
                nc.scalar.copy(projq[:, at, :], pq_ps[:, :BPC])

            # ---- epilogue pieces -------------------------------------
            def emit_scores(pts):
                """Col-tiled scores: ONE PSUM bank, 4 stripes at partitions
                {0,32,64,96}; start=False onto a zeroed bank; whole-bank exp
                in one ACT op."""
                sps = sc_ps.tile([P, SF], F32, name="sps")
                nc.vector.memset(sps, 0.0)
                for at in range(AT):
                    for sc in range(SC):
                        nc.tensor.matmul(
                            sps[32 * sc : 32 * sc + 1, :],
                            watt_sb[:, at : at + 1],
                            pts[at][:, sc * SF : (sc + 1) * SF],
                            start=False,
                            stop=(at == AT - 1),
                            skip_group_check=True,
                            tile_position=(0, 32 * sc),
                        )
                exp_sb = small.tile([P, SF], BF, name="exp_sb")
                esum = small.tile([P, 1], F32, name="esum")
                nc.scalar.activation(exp_sb, sps, EXP, accum_out=esum)
                return exp_sb, esum

            def emit_wbc_dma(exp_sb):
                # gather the 4 stripe rows into DRAM, broadcast back to all
                # 128 partitions (stride-0 read); sync ring (shared with x
                # loads - measured fine in v2)
                exp_dram = dram.tile([1, S], BF, name="exp_dram")
                nc.sync.dma_start(
                    bass.AP(
                        tensor=exp_dram.tensor,
                        offset=exp_dram.offset,
                        ap=[[SF, SC], [1, SF]],
                    ),
                    exp_sb[0 : 32 * SC - 31 : 32, :],
                )
                wbc = ttp.tile([P, S], BF, name="wbc")
                nc.sync.dma_start(
                    wbc,
                    bass.AP(
                        tensor=exp_dram.tensor,
                        offset=exp_dram.offset,
                        ap=[[0, P], [1, S]],
                    ),
                )
                return wbc

            def emit_tot_rcp(wbc):
                tot = small.tile([P, 1], F32, name="tot")
                nc.vector.tensor_reduce(
                    tot, wbc, axis=mybir.AxisListType.X, op=mybir.AluOpType.add
                )
                rcp = small.tile([P, 1], F32, name="rcp")
                nc.vector.reciprocal(rcp, tot)
                return rcp

            # ---- main batch loop -------------------------------------
            prev = None  # (batch_idx, t tiles, projTall)
            ep = {}  # in-flight deferred epilogue state
            for b in range(BPC):
                last = b == BPC - 1
                # quad tiles [P,4,S]; batch 0 fills each with two pair-DMAs
                # (finer arrival granularity under the startup DMA ramp),
                # later batches with one quad-DMA (fewer instructions ->
                # shorter end-of-kernel completion-poke drain)
                xquads = []
                for h in range(EC // 4):
                    xq = xtp.tile([P, 4, S], BF, name=f"xq{h}")
                    nparts = 2 if b == 0 else 1
                    step = 4 // nparts
                    for j in range(nparts):
                        nc.sync.dma_start(
                            xq[:, j * step : (j + 1) * step, :],
                            bass.AP(
                                tensor=xT,
                                offset=(b * E + (h * 4 + j * step) * P) * S,
                                ap=[[S, P], [P * S, step], [1, S]],
                            ),
                        )
                    xquads.append(xq)

                ts_ = []
                projTall = ttp.tile([P, AT * S], BF, name="projTall", bufs=3)
                for at in range(AT):
                    t_sb = ttp.tile([P, S], BF, name=f"t{at}")
                    pss = [mm_ps.tile([P, SF], F32, name="mm_acc") for _ in range(SC)]
                    for ec in range(EC):
                        for sc in range(SC):
                            nc.tensor.matmul(
                                pss[sc],
                                w_sb[:, ec, at, :],
                                xquads[ec // 4][:, ec % 4, sc * SF : (sc + 1) * SF],
                                start=(ec == 0),
                                stop=(ec == EC - 1),
                            )
                    for sc in range(SC):
                        sl = slice(at * S + sc * SF, at * S + (sc + 1) * SF)
                        if last and at == AT - 1:
                            # tail: tanh reads PSUM directly (bias fused) so
                            # the final scores aren't gated on the cast
                            # chain; the cast runs in parallel on DVE.
                            # PSUM-release timing is moot here (the banks
                            # are next used by the tail's own matmuls).
                            nc.scalar.activation(
                                t_sb[:, sc * SF : (sc + 1) * SF],
                                pss[sc],
                                TANH,
                                bias=projq[:, at, b : b + 1],
                            )
                            nc.vector.tensor_copy(projTall[:, sl], pss[sc])
                        else:
                            # single PSUM reader (DVE cast) gates PSUM
                            # release; tanh reads the SBUF copy (bias fused)
                            nc.vector.tensor_copy(projTall[:, sl], pss[sc])
                            nc.scalar.activation(
                                t_sb[:, sc * SF : (sc + 1) * SF],
                                projTall[:, sl],
                                TANH,
                                bias=projq[:, at, b : b + 1],
                            )
                    ts_.append(t_sb)

                    if at == 1 and prev is not None:
                        # deferred epilogue part 1: scores / exp / broadcast
                        ep["b"], ep["ts"], ep["proj"] = prev
                        ep["exp"], _ = emit_scores(ep["ts"])
                        ep["wbc"] = emit_wbc_dma(ep["exp"])
                        ep["cscr"] = [
                            ttp.tile([P, S], BF, name=f"cscr{i}", bufs=1)
                            for i in range(AT)
                        ]
                    if at == 2 and prev is not None:
                        # part 2a: first two ctx multiplies
                        for i in range(2):
                            nc.vector.tensor_tensor(
                                out=ep["cscr"][i],
                                in0=ep["proj"][:, i * S : (i + 1) * S],
                                in1=ep["wbc"],
                                op=mybir.AluOpType.mult,
                            )

                # part 2b: remaining mults + denominator (DVE); for b0/b1
                # carriers also the reduces/store; b2's reduces move past
                # the final scores/exp (delta 4)
                if prev is not None:
                    for i in range(2, AT):
                        nc.vector.tensor_tensor(
                            out=ep["cscr"][i],
                            in0=ep["proj"][:, i * S : (i + 1) * S],
                            in1=ep["wbc"],
                            op=mybir.AluOpType.mult,
                        )
                    ep["rcp"] = emit_tot_rcp(ep["wbc"])
                    ep["c"] = small.tile([P, AT], F32, name="c")
                    if not last:
                        for i in range(AT):
                            nc.scalar.activation(
                                ep["cscr"][i], ep["cscr"][i], COPY,
                                accum_out=ep["c"][:, i : i + 1],
                            )
                        for i in range(AT):
                            nc.gpsimd.tensor_scalar_mul(
                                ep["c"][:, i : i + 1], ep["c"][:, i : i + 1], ep["rcp"]
                            )
                        nc.scalar.dma_start(
                            bass.AP(
                                tensor=out, offset=ep["b"] * A, ap=[[1, P], [P, AT]]
                            ),
                            ep["c"],
                        )

                prev = (b, ts_, projTall)

            # ---- final epilogue (latency-critical, PE idle afterwards) --
            pb, pts, pproj = prev
            exp_sb, esum = emit_scores(pts)
            # denominator via tiny PE matmuls: tot = mask . esum, broadcast
            # with a K=1 ones matmul, reciprocal on [128,1] (delta 3)
            tot_ps = mm_ps.tile([P, SF], F32, name="mm_acc")
            nc.tensor.matmul(tot_ps[:1, :1], mask_f, esum, start=True, stop=True)
            tot_sb = small.tile([1, 1], F32, name="tot_sb")
            nc.scalar.copy(tot_sb, tot_ps[:1, :1])
            totbc_ps = mm_ps.tile([P, SF], F32, name="mm_acc")
            nc.tensor.matmul(totbc_ps[:, :1], ones_f, tot_sb, start=True, stop=True)
            # PE K=1 ones-matmul broadcast of the exp stripe rows
            wbc = ttp.tile([P, S], BF, name="wbc")
            wpss = []
            for sc in range(SC):
                wps = mm_ps.tile([P, SF], F32, name="mm_acc")
                nc.tensor.matmul(
                    wps,
                    ones2[32 * sc : 32 * sc + 1, :],
                    exp_sb[32 * sc : 32 * sc + 1, :],
                    start=True,
                    stop=True,
                    tile_position=(32 * sc, 0),
                )
                wpss.append(wps)
            for sc in range(SC):
                dst = wbc[:, sc * SF : (sc + 1) * SF]
                if sc % 2 == 0:
                    nc.vector.tensor_copy(dst, wpss[sc])
                else:
                    nc.scalar.copy(dst, wpss[sc])
            totbc = small.tile([P, 1], F32, name="totbc")
            nc.vector.tensor_copy(totbc, totbc_ps[:, :1])
            rcp = small.tile([P, 1], F32, name="rcp")
            nc.vector.reciprocal(rcp, totbc)

            # b2's reduces/normalize/store now fill the ACT idle slots
            # behind the final exp (delta 4)
            for i in range(2):
                nc.scalar.activation(
                    ep["cscr"][i], ep["cscr"][i], COPY, accum_out=ep["c"][:, i : i + 1]
                )

            # final ctx: first multiply chunked so it starts on the first
            # broadcast chunk; reduces split ACT/DVE (delta 5)
            cscrs = [ttp.tile([P, S], BF, name=f"cscr{i}", bufs=1) for i in range(AT)]
            c = small.tile([P, AT], F32, name="c")
            for sc in range(SC):
                sl = slice(sc * SF, (sc + 1) * SF)
                nc.vector.tensor_tensor(
                    out=cscrs[0][:, sl],
                    in0=pproj[:, sc * SF : (sc + 1) * SF],
                    in1=wbc[:, sl],
                    op=mybir.AluOpType.mult,
                )
            for i in (1, 2, 3):
                nc.vector.tensor_tensor(
                    out=cscrs[i],
                    in0=pproj[:, i * S : (i + 1) * S],
                    in1=wbc,
                    op=mybir.AluOpType.mult,
                )
            for i in range(2, AT):
                nc.scalar.activation(
                    ep["cscr"][i], ep["cscr"][i], COPY, accum_out=ep["c"][:, i : i + 1]
                )
            for i in range(AT):
                nc.gpsimd.tensor_scalar_mul(
                    ep["c"][:, i : i + 1], ep["c"][:, i : i + 1], ep["rcp"]
                )
            nc.scalar.dma_start(
                bass.AP(tensor=out, offset=ep["b"] * A, ap=[[1, P], [P, AT]]),
                ep["c"],
            )
            # r0/r2 whole-row on ACT; r1/r3 split: DVE folds the halves
            # (bf16 2x mode) then ACT reduces the 1024-wide result - the
            # full-width DVE reduce runs 1x and was the tail's last 4.5us
            nc.scalar.activation(cscrs[0], cscrs[0], COPY, accum_out=c[:, 0:1])
            nc.scalar.activation(cscrs[2], cscrs[2], COPY, accum_out=c[:, 2:3])
            for i in (1, 3):
                nc.vector.tensor_tensor(
                    out=cscrs[i][:, : S // 2],
                    in0=cscrs[i][:, : S // 2],
                    in1=cscrs[i][:, S // 2 :],
                    op=mybir.AluOpType.add,
                )
                nc.scalar.activation(
                    cscrs[i][:, : S // 2], cscrs[i][:, : S // 2], COPY,
                    accum_out=c[:, i : i + 1],
                )
            for at in range(AT):
                nc.vector.tensor_scalar_mul(c[:, at : at + 1], c[:, at : at + 1], rcp)
            nc.sync.dma_start(
                bass.AP(tensor=out, offset=pb * A, ap=[[1, P], [P, AT]]),
                c,
            )

    nc.compile()
    return nc


_nc = None


def kernel(inputs, query, W_in, W_q, w_att):
    global _nc
    if _nc is None:
        _nc = build()

    bf = ml_dtypes.bfloat16
    x_bf = np.asarray(inputs).astype(bf)
    xT_bf = np.ascontiguousarray(x_bf.transpose(0, 2, 1))
    w_in_bf = np.ascontiguousarray(np.asarray(W_in).astype(bf))
    w_q_bf = np.ascontiguousarray(np.asarray(W_q).astype(bf))
    w_att_bf = np.ascontiguousarray(np.asarray(w_att).astype(bf))

    in_maps = []
    for c in range(NCORES):
        sl = slice(c * BPC, (c + 1) * BPC)
        qTp = np.zeros((Q, QPAD), dtype=bf)
        qTp[:, :BPC] = np.asarray(query[sl]).astype(bf).T
        in_maps.append(
            {
                "xT": np.ascontiguousarray(xT_bf[sl]),
                "qT": qTp,
                "w_in": w_in_bf,
                "w_q": w_q_bf,
                "w_att": w_att_bf,
            }
        )

    res = bass_utils.run_bass_kernel_spmd(_nc, in_maps, core_ids=list(range(NCORES)))
    return np.concatenate([r["out"] for r in res.results], axis=0)


if __name__ == "__main__":
    rng = np.random.default_rng(0)
    ins = {
        "inputs": rng.standard_normal((B, S, E), dtype=np.float32),
        "query": rng.standard_normal((B, Q), dtype=np.float32),
        "W_in": (rng.standard_normal((E, A), dtype=np.float32) / np.sqrt(E)).astype(
            np.float32
        ),
        "W_q": (rng.standard_normal((Q, A), dtype=np.float32) / np.sqrt(Q)).astype(
            np.float32
        ),
        "w_att": (rng.standard_normal((A,), dtype=np.float32) / np.sqrt(A)).astype(
            np.float32
        ),
    }
    got = kernel(**ins)
    print("out shape", got.shape, got.dtype)


# revision 23
# speedup vs baseline: 1.3408x; 1.0127x over previous
"""Bahdanau (additive) attention TRN2 Bass kernel (v6, 168.5us).

reference:
    proj_in = einsum("bse,ea->bsa", inputs, W_in)      # [B,S,A]
    proj_q  = (query @ W_q)[:, None, :]                # [B,1,A]
    scores  = einsum("bsa,a->bs", tanh(proj_in+proj_q), w_att)
    weights = softmax(scores, axis=1)
    context = einsum("bs,bsa->ba", weights, proj_in)   # [B,A]

B,S,E,Q,A = 32,2048,1024,1024,512.

Sharding: data-parallel over batch. 8 cores x 4 batches each; weights
replicated. No collectives; host scatters inputs / gathers outputs.

v6 = the measured-best v2 structure (178.3us) + local deltas
(baseline 188.4us -> v2 178.3 -> v5b 169.5 -> v6 168.5):
  1. qT/wq/watt loads moved from the gpsimd ring (software DGE - they
     completed at ~17us and stalled the warmup) to the HEAD of the sync
     hardware ring, before the x loads.  w_in stays on the scalar ring.
  2. x loaded as pair-DMAs [128,2,S]: 16 instead of 32 DMA instructions
     (the end-of-kernel drain serially retires ~16 completion pokes per
     DMA instruction at ~20ns).
  3. Final softmax denominator via tiny PE matmuls (mask-dot of the exp
     accumulator + K=1 ones broadcast) instead of a 2.27us DVE reduce
     sitting in the tail's critical DVE chain.
  4. The last deferred epilogue's ACT reduces are emitted AFTER the
     final scores/exp so they fill ACT idle slots instead of delaying
     the final exp (v2 measured a 3.4us exp wait).
  5. Final ctx reduces split ACT/DVE; first ctx multiply chunked so it
     starts on the first broadcast chunk.

Carried over from v2 (all trace-validated):
  - Main loop (at, ec-outer, sc-inner), stationary reused, mm_acc 6
    PSUM banks; per-MM spacing is 216ns (LDWEIGHTS fully hidden).
  - Scores col-tiled into ONE PSUM bank: stripes at partitions
    {0,32,64,96} via tile_position, memset + start=False accumulation
    (no whole-bank has_written-clear hazard), measured concurrent
    (4 MMs per ~390ns).  Whole-bank exp in one ACT op.
  - Softmax denominator for deferred epilogues from a DVE reduce of the
    broadcast wbc (every partition computes the total).
  - Deferred epilogue split across the carrier's at-groups; epilogue
    normalize-muls on GPSIMD, stores on the scalar ring, so the DVE
    queue (PSUM-release casts) and sync queue (x loads) never block.
  - Final epilogue: PE K=1 ones-matmul broadcast from the stripe rows,
    drains ping-ponged DVE/ACT.
  - proj_q warmup baseline-style (per-at, wq stationary, no DRAM
    bounce - a bounced bias picked up spurious queue dependencies and
    starved every tanh for 45us); doubles as the HAM warmup.
"""

import sys

sys.path.insert(0, "/opt/trn_rl_repo")

import ml_dtypes
import numpy as np

import concourse.bass as bass
import concourse.tile as tile
from concourse import bacc, bass_utils, mybir

B, S, E, Q, A = 32, 2048, 1024, 1024, 512
NCORES = 8
BPC = B // NCORES  # batches per core
P = 128
EC = E // P  # 8 e-chunks
QC = Q // P  # 8 q-chunks
AT = A // P  # 4 a-tiles
SF = 512  # matmul moving free dim
SC = S // SF  # 4 s-chunks
QPAD = 128  # padded free dim for the transposed proj_q warmup

BF = mybir.dt.bfloat16
F32 = mybir.dt.float32
TANH = mybir.ActivationFunctionType.Tanh
EXP = mybir.ActivationFunctionType.Exp
COPY = mybir.ActivationFunctionType.Copy


def build():
    nc = bacc.Bacc("TRN2", target_bir_lowering=False, debug=False)

    # qT/w_q/w_in are HOST-PRE-ARRANGED into their SBUF layouts so each
    # load is one big contiguous-line DMA (the [Q,*]-layout loads read
    # 256B-1KB strided lines and arrived ~12us late in the v6 trace)
    xT = nc.dram_tensor("xT", [BPC, E, S], BF, kind="ExternalInput")
    qT = nc.dram_tensor("qT", [P, QC, QPAD], BF, kind="ExternalInput")
    w_in = nc.dram_tensor("w_in", [P, EC, AT, P], BF, kind="ExternalInput")
    w_q = nc.dram_tensor("w_q", [P, QC, A], BF, kind="ExternalInput")
    w_att = nc.dram_tensor("w_att", [A], BF, kind="ExternalInput")
    out = nc.dram_tensor("out", [BPC, A], F32, kind="ExternalOutput")

    with tile.TileContext(nc) as tc:
        with (
            tc.tile_pool(name="const", bufs=1) as const,
            tc.tile_pool(name="xtp", bufs=2) as xtp,
            tc.tile_pool(name="ttp", bufs=2) as ttp,
            tc.tile_pool(name="small", bufs=3) as small,
            tc.tile_pool(name="mm_ps", bufs=7, space="PSUM") as mm_ps,
            tc.tile_pool(name="sc_ps", bufs=1, space="PSUM") as sc_ps,
            tc.tile_pool(name="dram", bufs=2, space="DRAM") as dram,
        ):
            # ---- weights: sync-ring head (hardware DGE); w_in on scalar.
            # All three are pre-arranged on the host: contiguous 2-8KB
            # partition lines, one DMA each.
            qT_sb = const.tile([P, QC, QPAD], BF)
            nc.sync.dma_start(qT_sb, qT.ap())
            wq_sb = const.tile([P, QC, A], BF)
            nc.sync.dma_start(wq_sb, w_q.ap())
            watt_sb = const.tile([P, AT], BF)
            nc.sync.dma_start(watt_sb, w_att.ap().rearrange("(at p) -> p at", p=P))

            w_sb = const.tile([P, EC, AT, P], BF)
            nc.scalar.dma_start(w_sb, w_in.ap())

            ones2 = const.tile([P, P], BF)
            nc.vector.memset(ones2, 1.0)
            ones_f = const.tile([1, P], F32)
            nc.vector.memset(ones_f, 1.0)
            mask_f = const.tile([P, 1], F32)
            nc.vector.memset(mask_f, 0.0)
            for sc in range(SC):
                nc.vector.memset(mask_f[32 * sc : 32 * sc + 1, :], 1.0)

            # ---- proj_q warmup, baseline-style (no DRAM bounce - a
            # bounced bias acquired spurious queue dependencies and
            # starved every tanh): per-at accumulation with wq stationary,
            # qT moving; PSUM copied straight to the per-at bias tiles.
            # 32 N=128 MMs also serve as the HAM warmup.
            projq = const.tile([P, AT, BPC], F32)
            for at in range(AT):
                pq_ps = mm_ps.tile([P, SF], F32, name="mm_acc")
                for qc in range(QC):
                    nc.tensor.matmul(
                        pq_ps[:, :QPAD],
                        wq_sb[:, qc, at * P : (at + 1) * P],
                        qT_sb[:, qc, :],
                        start=(qc == 0),
                        stop=(qc == QC - 1),
                    )
                nc.scalar.copy(projq[:, at, :], pq_ps[:, :BPC])

            # ---- epilogue pieces -------------------------------------
            def emit_scores(pts):
                """Col-tiled scores: ONE PSUM bank, 4 stripes at partitions
                {0,32,64,96}; start=False onto a zeroed bank; whole-bank exp
                in one ACT op."""
                sps = sc_ps.tile([P, SF], F32, name="sps")
                nc.vector.memset(sps, 0.0)
                for at in range(AT):
                    for sc in range(SC):
                        nc.tensor.matmul(
                            sps[32 * sc : 32 * sc + 1, :],
                            watt_sb[:, at : at + 1],
                            pts[at][:, sc * SF : (sc + 1) * SF],
                            start=False,
                            stop=(at == AT - 1),
                            skip_group_check=True,
                            tile_position=(0, 32 * sc),
                        )
                exp_sb = small.tile([P, SF], BF, name="exp_sb")
                esum = small.tile([P, 1], F32, name="esum")
                nc.scalar.activation(exp_sb, sps, EXP, accum_out=esum)
                return exp_sb, esum

            def emit_wbc_dma(exp_sb):
                # gather the 4 stripe rows into DRAM, broadcast back to all
                # 128 partitions (stride-0 read); sync ring (shared with x
                # loads - measured fine in v2)
                exp_dram = dram.tile([1, S], BF, name="exp_dram")
                nc.sync.dma_start(
                    bass.AP(
                        tensor=exp_dram.tensor,
                        offset=exp_dram.offset,
                        ap=[[SF, SC], [1, SF]],
                    ),
                    exp_sb[0 : 32 * SC - 31 : 32, :],
                )
                wbc = ttp.tile([P, S], BF, name="wbc")
                nc.sync.dma_start(
                    wbc,
                    bass.AP(
                        tensor=exp_dram.tensor,
                        offset=exp_dram.offset,
                        ap=[[0, P], [1, S]],
                    ),
                )
                return wbc

            def emit_tot_rcp(wbc):
                tot = small.tile([P, 1], F32, name="tot")
                nc.vector.tensor_reduce(
                    tot, wbc, axis=mybir.AxisListType.X, op=mybir.AluOpType.add
                )
                rcp = small.tile([P, 1], F32, name="rcp")
                nc.vector.reciprocal(rcp, tot)
                return rcp

            # ---- main batch loop -------------------------------------
            prev = None  # (batch_idx, t tiles, projTall)
            ep = {}  # in-flight deferred epilogue state
            for b in range(BPC):
                last = b == BPC - 1
                # quad tiles [P,4,S]; batch 0 fills each with two pair-DMAs
                # (finer arrival granularity under the startup DMA ramp),
                # later batches with one quad-DMA (fewer instructions ->
                # shorter end-of-kernel completion-poke drain)
                xquads = []
                for h in range(EC // 4):
                    xq = xtp.tile([P, 4, S], BF, name=f"xq{h}")
                    nparts = 2 if b == 0 else 1
                    step = 4 // nparts
                    for j in range(nparts):
                        nc.sync.dma_start(
                            xq[:, j * step : (j + 1) * step, :],
                            bass.AP(
                                tensor=xT,
                                offset=(b * E + (h * 4 + j * step) * P) * S,
                                ap=[[S, P], [P * S, step], [1, S]],
                            ),
                        )
                    xquads.append(xq)

                ts_ = []
                projTall = ttp.tile([P, AT * S], BF, name="projTall", bufs=3)
                for at in range(AT):
                    t_sb = ttp.tile([P, S], BF, name=f"t{at}")
                    pss = [mm_ps.tile([P, SF], F32, name="mm_acc") for _ in range(SC)]
                    for ec in range(EC):
                        for sc in range(SC):
                            nc.tensor.matmul(
                                pss[sc],
                                w_sb[:, ec, at, :],
                                xquads[ec // 4][:, ec % 4, sc * SF : (sc + 1) * SF],
                                start=(ec == 0),
                                stop=(ec == EC - 1),
                            )
                    for sc in range(SC):
                        sl = slice(at * S + sc * SF, at * S + (sc + 1) * SF)
                        if last and at == AT - 1:
                            # tail: tanh reads PSUM directly (bias fused) so
                            # the final scores aren't gated on the cast
                            # chain; the cast runs in parallel on DVE.
                            # PSUM-release timing is moot here (the banks
                            # are next used by the tail's own matmuls).
                            nc.scalar.activation(
                                t_sb[:, sc * SF : (sc + 1) * SF],
                                pss[sc],
                                TANH,
                                bias=projq[:, at, b : b + 1],
                            )
                            nc.vector.tensor_copy(projTall[:, sl], pss[sc])
                        else:
                            # single PSUM reader (DVE cast) gates PSUM
                            # release; tanh reads the SBUF copy (bias fused)
                            nc.vector.tensor_copy(projTall[:, sl], pss[sc])
                            nc.scalar.activation(
                                t_sb[:, sc * SF : (sc + 1) * SF],
                                projTall[:, sl],
                                TANH,
                                bias=projq[:, at, b : b + 1],
                            )
                    ts_.append(t_sb)

                    if at == 1 and prev is not None:
                        # deferred epilogue part 1: scores / exp / broadcast
                        ep["b"], ep["ts"], ep["proj"] = prev
                        ep["exp"], _ = emit_scores(ep["ts"])
                        ep["wbc"] = emit_wbc_dma(ep["exp"])
                        ep["cscr"] = [
                            ttp.tile([P, S], BF, name=f"cscr{i}", bufs=1)
                            for i in range(AT)
                        ]
                    if at == 2 and prev is not None:
                        # part 2a: first two ctx multiplies
                        for i in range(2):
                            nc.vector.tensor_tensor(
                                out=ep["cscr"][i],
                                in0=ep["proj"][:, i * S : (i + 1) * S],
                                in1=ep["wbc"],
                                op=mybir.AluOpType.mult,
                            )

                # part 2b: remaining mults + denominator (DVE); for b0/b1
                # carriers also the reduces/store; b2's reduces move past
                # the final scores/exp (delta 4)
                if prev is not None:
                    for i in range(2, AT):
                        nc.vector.tensor_tensor(
                            out=ep["cscr"][i],
                            in0=ep["proj"][:, i * S : (i + 1) * S],
                            in1=ep["wbc"],
                            op=mybir.AluOpType.mult,
                        )
                    ep["rcp"] = emit_tot_rcp(ep["wbc"])
                    ep["c"] = small.tile([P, AT], F32, name="c")
                    if not last:
                        for i in range(AT):
                            nc.scalar.activation(
                                ep["cscr"][i], ep["cscr"][i], COPY,
                                accum_out=ep["c"][:, i : i + 1],
                            )
                        for i in range(AT):
                            nc.gpsimd.tensor_scalar_mul(
                                ep["c"][:, i : i + 1], ep["c"][:, i : i + 1], ep["rcp"]
                            )
                        nc.scalar.dma_start(
                            bass.AP(
                                tensor=out, offset=ep["b"] * A, ap=[[1, P], [P, AT]]
                            ),
                            ep["c"],
                        )

                prev = (b, ts_, projTall)

            # ---- final epilogue (latency-critical, PE idle afterwards) --
            pb, pts, pproj = prev
            exp_sb, esum = emit_scores(pts)
            # denominator via tiny PE matmuls: tot = mask . esum, broadcast
            # with a K=1 ones matmul, reciprocal on [128,1] (delta 3)
            tot_ps = mm_ps.tile([P, SF], F32, name="mm_acc")
            nc.tensor.matmul(tot_ps[:1, :1], mask_f, esum, start=True, stop=True)
            tot_sb = small.tile([1, 1], F32, name="tot_sb")
            nc.scalar.copy(tot_sb, tot_ps[:1, :1])
            totbc_ps = mm_ps.tile([P, SF], F32, name="mm_acc")
            nc.tensor.matmul(totbc_ps[:, :1], ones_f, tot_sb, start=True, stop=True)
            # PE K=1 ones-matmul broadcast of the exp stripe rows
            wbc = ttp.tile([P, S], BF, name="wbc")
            wpss = []
            for sc in range(SC):
                wps = mm_ps.tile([P, SF], F32, name="mm_acc")
                nc.tensor.matmul(
                    wps,
                    ones2[32 * sc : 32 * sc + 1, :],
                    exp_sb[32 * sc : 32 * sc + 1, :],
                    start=True,
                    stop=True,
                    tile_position=(32 * sc, 0),
                )
                wpss.append(wps)
            for sc in range(SC):
                dst = wbc[:, sc * SF : (sc + 1) * SF]
                if sc % 2 == 0:
                    nc.vector.tensor_copy(dst, wpss[sc])
                else:
                    nc.scalar.copy(dst, wpss[sc])
            totbc = small.tile([P, 1], F32, name="totbc")
            nc.vector.tensor_copy(totbc, totbc_ps[:, :1])
            rcp = small.tile([P, 1], F32, name="rcp")
            nc.vector.reciprocal(rcp, totbc)

            # b2's reduces/normalize/store now fill the ACT idle slots
            # behind the final exp (delta 4)
            for i in range(2):
                nc.scalar.activation(
                    ep["cscr"][i], ep["cscr"][i], COPY, accum_out=ep["c"][:, i : i + 1]
                )

            # final ctx: first multiply chunked so it starts on the first
            # broadcast chunk; reduces split ACT/DVE (delta 5)
            cscrs = [ttp.tile([P, S], BF, name=f"cscr{i}", bufs=1) for i in range(AT)]
            c = small.tile([P, AT], F32, name="c")
            for sc in range(SC):
                sl = slice(sc * SF, (sc + 1) * SF)
                nc.vector.tensor_tensor(
                    out=cscrs[0][:, sl],
                    in0=pproj[:, sc * SF : (sc + 1) * SF],
                    in1=wbc[:, sl],
                    op=mybir.AluOpType.mult,
                )
            for i in (1, 2, 3):
                nc.vector.tensor_tensor(
                    out=cscrs[i],
                    in0=pproj[:, i * S : (i + 1) * S],
                    in1=wbc,
                    op=mybir.AluOpType.mult,
                )
            for i in range(2, AT):
                nc.scalar.activation(
                    ep["cscr"][i], ep["cscr"][i], COPY, accum_out=ep["c"][:, i : i + 1]
                )
            for i in range(AT):
                nc.gpsimd.tensor_scalar_mul(
                    ep["c"][:, i : i + 1], ep["c"][:, i : i + 1], ep["rcp"]
                )
            nc.scalar.dma_start(
                bass.AP(tensor=out, offset=ep["b"] * A, ap=[[1, P], [P, AT]]),
                ep["c"],
            )
            # r0/r2 whole-row on ACT; r1/r3 split: DVE folds the halves
            # (bf16 2x mode) then ACT reduces the 1024-wide result - the
            # full-width DVE reduce runs 1x and was the tail's last 4.5us
            nc.scalar.activation(cscrs[0], cscrs[0], COPY, accum_out=c[:, 0:1])
            nc.scalar.activation(cscrs[2], cscrs[2], COPY, accum_out=c[:, 2:3])
            for i in (1, 3):
                nc.vector.tensor_tensor(
                    out=cscrs[i][:, : S // 2],
                    in0=cscrs[i][:, : S // 2],
                    in1=cscrs[i][:, S // 2 :],
                    op=mybir.AluOpType.add,
                )
                nc.scalar.activation(
                    cscrs[i][:, : S // 2], cscrs[i][:, : S // 2], COPY,
                    accum_out=c[:, i : i + 1],
                )
            for at in range(AT):
                nc.vector.tensor_scalar_mul(c[:, at : at + 1], c[:, at : at + 1], rcp)
            # scalar ring: lets the sync ring quiesce early so its
            # completion-poke backlog drains during the tail compute
            nc.scalar.dma_start(
                bass.AP(tensor=out, offset=pb * A, ap=[[1, P], [P, AT]]),
                c,
            )

    nc.compile()
    return nc


_nc = None


def prep_in_maps(inputs, query, W_in, W_q, w_att):
    """Host-side shard + pre-layout: x transposed to [b,e,s]; qT/w_q/w_in
    rearranged into their SBUF layouts so device loads are contiguous."""
    bf = ml_dtypes.bfloat16
    x_bf = np.asarray(inputs).astype(bf)
    xT_bf = np.ascontiguousarray(x_bf.transpose(0, 2, 1))
    # w_in[e, a] -> [p, ec, at, j] with e = ec*128+p, a = at*128+j
    w_in_pre = np.ascontiguousarray(
        np.asarray(W_in).astype(bf).reshape(EC, P, AT, P).transpose(1, 0, 2, 3)
    )
    # w_q[q, a] -> [p, qc, a] with q = qc*128+p
    w_q_pre = np.ascontiguousarray(
        np.asarray(W_q).astype(bf).reshape(QC, P, A).transpose(1, 0, 2)
    )
    w_att_bf = np.ascontiguousarray(np.asarray(w_att).astype(bf))

    in_maps = []
    for c in range(NCORES):
        sl = slice(c * BPC, (c + 1) * BPC)
        qTp = np.zeros((Q, QPAD), dtype=bf)
        qTp[:, :BPC] = np.asarray(query[sl]).astype(bf).T
        # qT[q, b_pad] -> [p, qc, b_pad]
        qT_pre = np.ascontiguousarray(
            qTp.reshape(QC, P, QPAD).transpose(1, 0, 2)
        )
        in_maps.append(
            {
                "xT": np.ascontiguousarray(xT_bf[sl]),
                "qT": qT_pre,
                "w_in": w_in_pre,
                "w_q": w_q_pre,
                "w_att": w_att_bf,
            }
        )
    return in_maps


def kernel(inputs, query, W_in, W_q, w_att):
    global _nc
    if _nc is None:
        _nc = build()
    in_maps = prep_in_maps(inputs, query, W_in, W_q, w_att)
    res = bass_utils.run_bass_kernel_spmd(_nc, in_maps, core_ids=list(range(NCORES)))
    return np.concatenate([r["out"] for r in res.results], axis=0)


if __name__ == "__main__":
    rng = np.random.default_rng(0)
    ins = {
        "inputs": rng.standard_normal((B, S, E), dtype=np.float32),
        "query": rng.standard_normal((B, Q), dtype=np.float32),
        "W_in": (rng.standard_normal((E, A), dtype=np.float32) / np.sqrt(E)).astype(
            np.float32
        ),
        "W_q": (rng.standard_normal((Q, A), dtype=np.float32) / np.sqrt(Q)).astype(
            np.float32
        ),
        "w_att": (rng.standard_normal((A,), dtype=np.float32) / np.sqrt(A)).astype(
            np.float32
        ),
    }
    got = kernel(**ins)
    print("out shape", got.shape, got.dtype)
